# revision 1
# baseline (speedup 1.0000x reference)
"""OCS fused kernel for Trainium2, data-parallel over batch across 8 cores.

Algebraic restructuring (verified vs reference to ~1e-6 in fp64):

Spatial branch (4 scan orders, shared weights) collapses to a symmetric
5-point stencil with scan-order wrap rules, and the two 1x1 convs fold
through it:
    W_proj @ y_sp = A2 @ sx + (B3 - W_proj) @ x
    sx = sum of 4 flat shifts of x (+/-1 row-major, +/-w) + col-scan wraps
Channel branch: m = g g^T is rank-1, so the whole conv pipeline collapses
into three [32,128] matmuls on shifted x (weights MP/MQ/MR = u (x) P/Q/R
built on-device from g = mean of x), a silu, and one [128,32] matmul.
Diff branch: |x - nb| terms are shared between opposite directions, so one
|dx| array per axis + shifted adds gives the 4-neighbor abs-diff sum S;
W_proj folds in as W_d @ S.
BatchNorm: per-core partial (sum, sumsq) -> 1KB AllReduce -> affine.
"""

import numpy as np
import ml_dtypes

B, C, Himg, Wimg = 8, 128, 128, 128
L = Himg * Wimg            # 16384
NCORES = 8
NCH = 512                  # psum chunk columns
NCHUNK = L // NCH          # 32
NW = 2048                  # elementwise window columns (4 chunks)
NGRP = L // NW             # 8
EPS_BN = 1e-5
NTOT = float(B * L)        # batchnorm population per channel

_CACHE = {}


def _make_patched_tc():
    """TileContext whose exit drain splits sem waits one-per-Drain.

    The walrus build in this container rejects Drain instructions carrying
    more than one sem wait ("Too many sync wait commands"). Stock
    TileContext attaches the whole global vector clock to a single tail
    Drain; emit one Drain per outstanding proc instead.
    """
    import bass_rust
    import concourse.tile as tile
    from concourse.vector_clock import ScopedClock

    class PatchedTC(tile.TileContext):
        def _drain_and_barrier(self, tick_clock, wait_clock):
            gc = list(tick_clock.global_clock)
            for i, v in enumerate(gc):
                if v:
                    single = [0] * len(gc)
                    single[i] = v
                    d = self.nc.sync.drain()
                    wait_clock.add_sem_waits(
                        d.ins, ScopedClock({None: bass_rust.VectorClock(single)})
                    )
            self.nc.all_engine_barrier()
            assert self.sems is not None
            popped = self.nc._tile_sem_poison_stack.pop()
            assert popped is self._sem_poison
            self.nc.clear_and_free_semaphores(list(self.sems.allocated().values()))
            self.nc.all_engine_barrier()

    return PatchedTC


def _split_excess_waits(nc):
    """Walrus here allows one sem wait per instruction; hoist extras onto
    same-engine NoOps inserted immediately before the instruction."""
    import bass_rust

    nid = 0
    for blk in nc.main_func.blocks:
        out = []
        for ins in blk.instructions:
            si = getattr(ins, "sync_info", None)
            waits = list(si.on_wait) if si is not None else []
            if len(waits) > 1:
                for w in waits[:-1]:
                    nid += 1
                    nop = bass_rust.InstNoOp(
                        name=f"I-waitsplit-{nid}", ins=[], outs=[])
                    nop.engine = ins.engine
                    nop.sync_info = bass_rust.SyncInfo(
                        on_wait=[w], on_update=[])
                    nc.register_instruction(nop, overwrite=True)
                    out.append(nop)
                si.on_wait = [waits[-1]]
                ins.sync_info = si
            out.append(ins)
        blk.instructions = out


def _build_program():
    import concourse.bass as bass
    import concourse.mybir as mybir

    PatchedTC = _make_patched_tc()

    f32 = mybir.dt.float32
    bf16 = mybir.dt.bfloat16
    Alu = mybir.AluOpType
    Act = mybir.ActivationFunctionType

    nc = bass.Bass(target_bir_lowering=False, num_devices=NCORES)

    x_ext = nc.declare_dram_parameter("x", [C, L], bf16, isOutput=False)
    wb3t_ext = nc.declare_dram_parameter("wb3t", [C, C], bf16, isOutput=False)
    wa2t_ext = nc.declare_dram_parameter("wa2t", [C, C], bf16, isOutput=False)
    wdt_ext = nc.declare_dram_parameter("wdt", [C, C], bf16, isOutput=False)
    c2t4_ext = nc.declare_dram_parameter("c2t4", [C, C], bf16, isOutput=False)
    wcho_ext = nc.declare_dram_parameter("wcho", [C, C], f32, isOutput=False)
    wchi_ext = nc.declare_dram_parameter("wchi", [C, C], f32, isOutput=False)
    wm1t_ext = nc.declare_dram_parameter("wm1t", [C, 32], f32, isOutput=False)
    taps_ext = nc.declare_dram_parameter("taps", [C, 3], f32, isOutput=False)
    b1t_ext = nc.declare_dram_parameter("b1t", [C, 1], f32, isOutput=False)
    bout_ext = nc.declare_dram_parameter("bout", [C, 1], f32, isOutput=False)
    gb_ext = nc.declare_dram_parameter("gb", [C, 2], f32, isOutput=False)
    y_ext = nc.declare_dram_parameter("y", [C, L], f32, isOutput=True)

    with PatchedTC(nc) as tc:
        with (
            tc.tile_pool(name="wp", bufs=1) as wp,
            tc.tile_pool(name="big", bufs=1) as big,
            tc.tile_pool(name="win", bufs=2) as win,
            tc.tile_pool(name="sm", bufs=1) as sm,
            tc.tile_pool(name="dump", bufs=2) as dump,
            tc.tile_pool(name="yps", bufs=5, space="PSUM") as yps,
            tc.tile_pool(name="hps", bufs=2, space="PSUM") as hps,
            tc.tile_pool(name="sps", bufs=1, space="PSUM") as sps,
            tc.tile_pool(name="dram", bufs=1, space="DRAM") as dram,
        ):
            # ---- weights to SBUF ----
            wb3t = wp.tile([C, C], bf16)
            wa2t = wp.tile([C, C], bf16)
            wdt = wp.tile([C, C], bf16)
            c2t4 = wp.tile([C, C], bf16)
            wcho = wp.tile([C, C], f32)
            wchi = wp.tile([C, C], f32)
            wm1t = wp.tile([C, 32], f32)
            taps = wp.tile([C, 3], f32)
            b1t = wp.tile([C, 1], f32)
            bout = wp.tile([C, 1], f32)
            gb = wp.tile([C, 2], f32)
            for t, e in [(wb3t, wb3t_ext), (wa2t, wa2t_ext), (wdt, wdt_ext),
                         (c2t4, c2t4_ext), (wcho, wcho_ext), (wchi, wchi_ext),
                         (wm1t, wm1t_ext), (taps, taps_ext), (b1t, b1t_ext),
                         (bout, bout_ext), (gb, gb_ext)]:
                nc.sync.dma_start(out=t, in_=e[:])

            # ---- big SBUF arrays ----
            xbf = big.tile([C, L], bf16)     # x (bf16, cast on host)
            ypre = big.tile([C, L], bf16)    # pre-BN output
            h1sb = big.tile([C, NGRP * NCH], bf16)  # silu(h1) packed 4ch/grp

            gsums = sm.tile([C, NGRP], f32)
            ysum = sm.tile([C, NCHUNK], f32)
            ysq = sm.tile([C, NCHUNK], f32)

            # ---- load x, accumulate row sums (dummy copy for accum) ----
            for g in range(NGRP):
                lo, hi = g * NW, (g + 1) * NW
                nc.sync.dma_start(out=xbf[:, lo:hi], in_=x_ext[:, lo:hi])
                gdump = dump.tile([C, NW], bf16, tag="gs")
                nc.scalar.activation(gdump, xbf[:, lo:hi], Act.Copy,
                                     accum_out=gsums[:, g:g + 1])

            # ---- elementwise diff windows ----
            Hws, Vws = [], []
            Shs, Svs = [], []
            for g in range(NGRP):
                G0 = g * NW
                sh = win.tile([C, NW], bf16, tag="sh")
                sv = win.tile([C, NW], bf16, tag="sv")
                # s_h[t] = x[l-1] + x[l+1]; s_v[t] = x[l-128] + x[l+128]
                ha = 1 if g == 0 else 0
                hb = NW - 1 if g == NGRP - 1 else NW
                nc.vector.tensor_tensor(sh[:, ha:hb],
                                        xbf[:, G0 + ha - 1:G0 + hb - 1],
                                        xbf[:, G0 + ha + 1:G0 + hb + 1],
                                        Alu.add)
                if g == 0:
                    nc.vector.tensor_copy(sh[:, 0:1], xbf[:, 1:2])
                if g == NGRP - 1:
                    nc.vector.tensor_copy(sh[:, NW - 1:NW],
                                          xbf[:, L - 2:L - 1])
                va = 128 if g == 0 else 0
                vb = NW - 128 if g == NGRP - 1 else NW
                nc.vector.tensor_tensor(sv[:, va:vb],
                                        xbf[:, G0 + va - 128:G0 + vb - 128],
                                        xbf[:, G0 + va + 128:G0 + vb + 128],
                                        Alu.add)
                if g == 0:
                    nc.vector.tensor_copy(sv[:, 0:128], xbf[:, 128:256])
                if g == NGRP - 1:
                    nc.vector.tensor_copy(sv[:, NW - 128:NW],
                                          xbf[:, L - NW - 128 + NW - 128:
                                               L - 128])
                Shs.append(sh)
                Svs.append(sv)
                dh = win.tile([C, NW + 4], bf16, tag="dh")
                dv = win.tile([C, NW + 128], bf16, tag="dv")
                Hw = win.tile([C, NW], bf16, tag="Hw")
                Vw = win.tile([C, NW], bf16, tag="Vw")
                Hws.append(Hw)
                Vws.append(Vw)

                # d_h[t] = |x[G0+t] - x[G0+t-1]|, t in [a, e)
                a = 1 if g == 0 else 0
                e = NW if g == NGRP - 1 else NW + 1
                nc.vector.tensor_tensor(dh[:, a:e], xbf[:, G0 + a:G0 + e],
                                        xbf[:, G0 + a - 1:G0 + e - 1],
                                        Alu.subtract)
                if g == 0:
                    nc.vector.memset(dh[:, 0:1], 0.0)
                dhu = dh.bitcast(mybir.dt.uint16)
                nc.vector.tensor_scalar(dhu[:, 0:e], dhu[:, 0:e], 0x7FFF,
                                        None, Alu.bitwise_and)
                # H[t] = d_h[t] + d_h[t+1], edges fixed per image row
                he = NW if g < NGRP - 1 else NW - 1
                nc.vector.tensor_tensor(Hw[:, 0:he], dh[:, 0:he], dh[:, 1:he + 1],
                                        Alu.add)
                h3 = Hw.rearrange("p (r c) -> p r c", c=Wimg)
                d3 = dh[:, 0:NW].rearrange("p (r c) -> p r c", c=Wimg)
                nc.vector.tensor_scalar(h3[:, :, 0:1], d3[:, :, 1:2], 2.0, None,
                                        Alu.mult)
                nc.vector.tensor_scalar(h3[:, :, Wimg - 1:Wimg],
                                        d3[:, :, Wimg - 1:Wimg], 2.0, None,
                                        Alu.mult)

                # d_v[t] = |x[G0+t] - x[G0+t-128]|, t in [av, ev)
                av = 128 if g == 0 else 0
                ev = NW if g == NGRP - 1 else NW + 128
                nc.vector.tensor_tensor(dv[:, av:ev], xbf[:, G0 + av:G0 + ev],
                                        xbf[:, G0 + av - 128:G0 + ev - 128],
                                        Alu.subtract)
                dvu = dv.bitcast(mybir.dt.uint16)
                nc.vector.tensor_scalar(dvu[:, av:ev], dvu[:, av:ev], 0x7FFF,
                                        None, Alu.bitwise_and)
                # V[t] = d_v[t] + d_v[t+128], first/last image row fixed
                vlo = 128 if g == 0 else 0
                vhi = NW - 128 if g == NGRP - 1 else NW
                nc.vector.tensor_tensor(Vw[:, vlo:vhi], dv[:, vlo:vhi],
                                        dv[:, vlo + 128:vhi + 128], Alu.add)
                if g == 0:
                    nc.vector.tensor_scalar(Vw[:, 0:128], dv[:, 128:256], 2.0,
                                            None, Alu.mult)
                if g == NGRP - 1:
                    nc.vector.tensor_scalar(Vw[:, NW - 128:NW],
                                            dv[:, NW - 128:NW], 2.0, None,
                                            Alu.mult)

            # ---- channel-branch small chain (needs all of x) ----
            gsum = sm.tile([C, 1], f32)
            nc.vector.tensor_reduce(gsum, gsums, mybir.AxisListType.X, Alu.add)
            ss_ps = sps.tile([1, 1], f32, tag="sp")
            nc.tensor.matmul(ss_ps, gsum, gsum, start=True, stop=True)
            ss = sm.tile([1, 1], f32)
            nc.vector.tensor_copy(ss, ss_ps)
            rn2 = sm.tile([1, 1], f32)
            nc.vector.reciprocal(rn2, ss)          # 1 / ||gsum||^2

            v_ps = sps.tile([C, 1], f32, tag="sp")
            nc.tensor.matmul(v_ps, wcho, gsum, start=True, stop=True)
            v_sb = sm.tile([C, 1], f32)
            nc.vector.tensor_copy(v_sb, v_ps)
            pqr = sm.tile([C, 3], f32)
            for j in range(3):
                nc.vector.tensor_tensor(pqr[:, j:j + 1], v_sb, taps[:, j:j + 1],
                                        Alu.mult)
            pqr2_ps = sps.tile([C, 3], f32, tag="sp")
            nc.tensor.matmul(pqr2_ps, wchi, pqr, start=True, stop=True)
            pqr2 = sm.tile([C, 3], f32)
            nc.vector.tensor_copy(pqr2, pqr2_ps)

            u_ps = sps.tile([1, 32], f32, tag="sp")
            nc.tensor.matmul(u_ps, gsum, wm1t, start=True, stop=True)
            u_sb = sm.tile([1, 32], f32)
            nc.vector.tensor_copy(u_sb, u_ps)
            u_sc = sm.tile([1, 32], f32)
            nc.vector.tensor_scalar(u_sc, u_sb, rn2[0:1, 0:1], None, Alu.mult)
            u_bc = sm.tile([C, 32], f32)
            u_dram = dram.tile([1, 32], f32)
            nc.sync.dma_start(out=u_dram[:], in_=u_sc)
            u_dram_bcast = bass.AP(
                tensor=u_dram.tensor, offset=u_dram.opt().offset,
                ap=[[0, C]] + list(u_dram.opt().ap[-1:]))
            nc.sync.dma_start(out=u_bc, in_=u_dram_bcast)

            mqt = sm.tile([C, 32], bf16)
            mpt = sm.tile([C, 32], bf16)
            mrt = sm.tile([C, 32], bf16)
            for t, j in [(mpt, 0), (mqt, 1), (mrt, 2)]:
                nc.vector.tensor_scalar(t, u_bc, pqr2[:, j:j + 1], None,
                                        Alu.mult)

            # ---- channel matmuls (col-tiled 4 chunks/bank) + silu ----
            for k in range(NGRP):
                h1ps = hps.tile([C, NCH], f32)
                for wgt, shift in [(mqt, 0), (mpt, -1), (mrt, +1)]:
                    for j in range(4):
                        n = 4 * k + j
                        n0 = n * NCH
                        lo = n0 + shift
                        hi = n0 + NCH + shift
                        plo, phi = 0, NCH
                        if lo < 0:
                            plo, lo = 1, 0
                        if hi > L:
                            phi, hi = NCH - 1, L
                        nc.tensor.matmul(
                            h1ps[32 * j:32 * j + 32, plo:phi],
                            wgt[:, 0:32], xbf[:, lo:hi],
                            start=(shift == 0), stop=(shift == 1),
                            tile_position=(0, 32 * j))
                nc.scalar.activation(h1sb[:, k * NCH:(k + 1) * NCH], h1ps,
                                     Act.Silu, bias=b1t[:, 0:1])

            # ---- main per-chunk accumulation ----
            # chunk 0 last: its wrap-correction reads the final x window
            for n in list(range(4, NCHUNK)) + list(range(4)):
                k, j = n // 4, n % 4
                n0 = n * NCH
                ps = yps.tile([C, NCH], f32)
                nc.tensor.matmul(ps, wb3t, xbf[:, n0:n0 + NCH],
                                 start=True, stop=False)
                # spatial shifts of x through A2 (pairs presummed on DVE)
                off0 = (n - 4 * k) * NCH
                nc.tensor.matmul(ps, wa2t, Shs[k][:, off0:off0 + NCH],
                                 start=False, stop=False)
                nc.tensor.matmul(ps, wa2t, Svs[k][:, off0:off0 + NCH],
                                 start=False, stop=False)
                if n == 0:
                    # col-scan wrap: l=j gets x[(h-1)w + j - 1]
                    nc.tensor.matmul(ps[:, 1:128], wa2t,
                                     xbf[:, L - Wimg:L - 1],
                                     start=False, stop=False)
                if n == NCHUNK - 1:
                    # col-scan wrap: l=(h-1)w+j gets x[j+1]
                    nc.tensor.matmul(ps[:, NCH - 128:NCH - 1], wa2t,
                                     xbf[:, 1:128], start=False, stop=False)
                # diff branch
                off = (n - 4 * k) * NCH
                nc.tensor.matmul(ps, wdt, Hws[k][:, off:off + NCH],
                                 start=False, stop=False)
                nc.tensor.matmul(ps, wdt, Vws[k][:, off:off + NCH],
                                 start=False, stop=False)
                # channel contribution (row-tiled, K=32)
                nc.tensor.matmul(ps, c2t4[32 * j:32 * j + 32, :],
                                 h1sb[32 * j:32 * j + 32, k * NCH:(k + 1) * NCH],
                                 start=False, stop=True,
                                 tile_position=(32 * j, 0))
                # evacuate + per-chunk channel sums
                nc.scalar.activation(ypre[:, n0:n0 + NCH], ps, Act.Identity,
                                     bias=bout[:, 0:1],
                                     accum_out=ysum[:, n:n + 1])
                # sum of squares straight off PSUM (note: Square ignores the
                # bias term; correct for it below via cross-term algebra)
                dmp = dump.tile([C, NCH], bf16, tag="sq")
                nc.scalar.activation(dmp, ps, Act.Square,
                                     accum_out=ysq[:, n:n + 1])

            # ---- global BN stats via AllReduce ----
            # ysq tracked z = y - bout (PSUM, pre-bias):
            #   sum(y^2) = sum(z^2) + 2*bout*sum(y) - L*bout^2
            stats = sm.tile([C, 2], f32)
            nc.vector.tensor_reduce(stats[:, 0:1], ysum, mybir.AxisListType.X,
                                    Alu.add)
            nc.vector.tensor_reduce(stats[:, 1:2], ysq, mybir.AxisListType.X,
                                    Alu.add)
            cb = sm.tile([C, 1], f32)
            nc.vector.tensor_tensor(cb, bout, stats[:, 0:1], Alu.mult)
            nc.vector.scalar_tensor_tensor(stats[:, 1:2], cb, 2.0,
                                           stats[:, 1:2], Alu.mult, Alu.add)
            bsq = sm.tile([C, 1], f32)
            nc.vector.tensor_tensor(bsq, bout, bout, Alu.mult)
            nc.vector.scalar_tensor_tensor(stats[:, 1:2], bsq, -float(L),
                                           stats[:, 1:2], Alu.mult, Alu.add)
            # prefetch the sqrt ACT table while the collective runs
            sqpre = sm.tile([C, 1], f32)
            nc.scalar.activation(sqpre, stats[:, 1:2], Act.Sqrt)
            cc_in = dram.tile([C, 2], f32)
            cc_out = dram.tile([C, 2], f32)
            nc.gpsimd.dma_start(out=cc_in[:], in_=stats)
            nc.gpsimd.collective_compute(
                "AllReduce", Alu.add,
                replica_groups=[list(range(NCORES))],
                ins=[cc_in.opt()], outs=[cc_out.opt()])
            statsr = sm.tile([C, 2], f32)
            nc.gpsimd.dma_start(out=statsr, in_=cc_out[:])

            mean = sm.tile([C, 1], f32)
            ex2 = sm.tile([C, 1], f32)
            nc.vector.tensor_scalar(mean, statsr[:, 0:1], 1.0 / NTOT, None,
                                    Alu.mult)
            nc.vector.tensor_scalar(ex2, statsr[:, 1:2], 1.0 / NTOT, None,
                                    Alu.mult)
            m2 = sm.tile([C, 1], f32)
            nc.vector.tensor_tensor(m2, mean, mean, Alu.mult)
            varep = sm.tile([C, 1], f32)
            nc.vector.tensor_tensor(varep, ex2, m2, Alu.subtract)
            nc.vector.tensor_scalar(varep, varep, EPS_BN, None, Alu.add)
            inv = sm.tile([C, 1], f32)
            nc.vector.reciprocal(inv, varep)
            rstd = sm.tile([C, 1], f32)
            nc.scalar.activation(rstd, inv, Act.Sqrt)
            s_sc = sm.tile([C, 1], f32)
            nc.vector.tensor_tensor(s_sc, rstd, gb[:, 0:1], Alu.mult)
            ms = sm.tile([C, 1], f32)
            nc.vector.tensor_tensor(ms, mean, s_sc, Alu.mult)
            t_sc = sm.tile([C, 1], f32)
            nc.vector.tensor_tensor(t_sc, gb[:, 1:2], ms, Alu.subtract)

            # ---- apply BN, write out (xb reused as f32 staging) ----
            for g in range(NGRP):
                lo, hi = g * NW, (g + 1) * NW
                ow = dump.tile([C, NW], f32, tag="ow")
                nc.vector.tensor_scalar(ow, ypre[:, lo:hi],
                                        s_sc[:, 0:1], t_sc[:, 0:1],
                                        Alu.mult, Alu.add)
                nc.sync.dma_start(out=y_ext[:, lo:hi], in_=ow)

    _split_excess_waits(nc)
    return nc


def _fold_weights(inputs):
    f = np.float32
    W_in = inputs["w_spatial_in"].astype(np.float64)
    W_out = inputs["w_spatial_out"].astype(np.float64)
    dw_sp = inputs["w_dw_spatial"][:, 0, :].astype(np.float64)
    W_proj = inputs["w_out_proj"].astype(np.float64)
    W_mlp2 = inputs["w_mlp2"].astype(np.float64)
    dwt = float(inputs["diff_weight"])

    a_sym = dw_sp[:, 0] + dw_sp[:, 2]
    w1 = dw_sp[:, 1]
    A2 = 0.25 * W_proj @ (W_out * a_sym[None, :]) @ W_in
    B3 = W_proj @ (W_out * w1[None, :]) @ W_in + W_proj
    W_d = 0.25 * dwt * W_proj
    C2 = W_proj @ W_mlp2                     # [c, 32]
    bias_out = W_proj @ inputs["b_mlp2"].astype(np.float64)

    bf = ml_dtypes.bfloat16
    return {
        "wb3t": np.ascontiguousarray(B3.T.astype(bf)),
        "wa2t": np.ascontiguousarray(A2.T.astype(bf)),
        "wdt": np.ascontiguousarray(W_d.T.astype(bf)),
        "c2t4": np.ascontiguousarray(np.tile(C2.T.astype(bf), (4, 1))),
        "wcho": np.ascontiguousarray(inputs["w_ch_out"].astype(f)),
        "wchi": np.ascontiguousarray(inputs["w_ch_in"].astype(f)),
        "wm1t": np.ascontiguousarray(inputs["w_mlp1"].T.astype(f)),
        "taps": np.ascontiguousarray(inputs["w_ch_dw"][:, 0, :].astype(f)),
        "b1t": np.ascontiguousarray(
            np.tile(inputs["b_mlp1"].astype(f), 4)[:, None]),
        "bout": np.ascontiguousarray(bias_out.astype(f)[:, None]),
        "gb": np.ascontiguousarray(
            np.stack([inputs["bn_gamma"], inputs["bn_beta"]], 1).astype(f)),
    }


def kernel(**inputs):
    from concourse.bass_utils import run_bass_kernel_spmd

    inputs = {k: np.asarray(v) for k, v in inputs.items()}
    if "nc" not in _CACHE:
        _CACHE["nc"] = _build_program()
    nc = _CACHE["nc"]

    wmap = _fold_weights(inputs)
    x = inputs["x"].astype(np.float32)  # [B, C, H, W]
    in_maps = []
    for b in range(NCORES):
        m = dict(wmap)
        m["x"] = np.ascontiguousarray(
            x[b].reshape(C, L).astype(ml_dtypes.bfloat16))
        in_maps.append(m)

    res = run_bass_kernel_spmd(nc, in_maps, list(range(NCORES)))
    out = np.stack([res.results[b]["y"].reshape(C, Himg, Wimg)
                    for b in range(NCORES)])
    return out.astype(np.float32)



# revision 3
# speedup vs baseline: 1.0945x; 1.0945x over previous
"""OCS fused kernel for Trainium2, data-parallel over batch across 8 cores.

Algebraic restructuring (verified vs reference to ~1e-6 in fp64):

Spatial branch (4 scan orders, shared weights) collapses to a symmetric
5-point stencil with scan-order wrap rules, and the two 1x1 convs fold
through it:
    W_proj @ y_sp = A2 @ (sxh + sxv) + (B3 - W_proj) @ x
    sxh/sxv = presummed +-1 / +-128 flat shifts of x; col-scan wraps are
    two extra small matmuls at the first/last chunk.
Channel branch: m = g g^T is rank-1, so the whole conv pipeline collapses
into three [32,128] matmuls on shifted x (weights MP/MQ/MR = u (x) P/Q/R
built on-device from g = sum of x), a silu, and one [128,32] matmul.
Diff branch: |dx| per axis once; the horizontal pair-sum H[l] =
|dh[l]|+|dh[l+1]| is ONE overlapping-pair tensor_reduce with
apply_absolute_value (4x-mode eligible); vertical V via bitand-abs + add.
W_proj folds in as W_d @ (H + V).
BatchNorm: per-core partial (sum, sumsq) -> 1KB AllReduce -> affine,
applied at DVE 2x with bf16 output (host upcasts to f32).
"""

import numpy as np
import ml_dtypes

B, C, Himg, Wimg = 8, 128, 128, 128
L = Himg * Wimg            # 16384
NCORES = 8
NCH = 512                  # psum chunk columns
NCHUNK = L // NCH          # 32
NW = 2048                  # elementwise window columns (4 chunks)
NGRP = L // NW             # 8
NROW = NW // Wimg          # image rows per window (16)
EPS_BN = 1e-5
NTOT = float(B * L)        # batchnorm population per channel

_CACHE = {}


def _make_patched_tc():
    """TileContext whose exit drain splits sem waits one-per-Drain.

    The walrus build in this container rejects Drain instructions carrying
    more than one sem wait ("Too many sync wait commands"). Stock
    TileContext attaches the whole global vector clock to a single tail
    Drain; emit one Drain per outstanding proc instead.
    """
    import bass_rust
    import concourse.tile as tile
    from concourse.vector_clock import ScopedClock

    class PatchedTC(tile.TileContext):
        def _drain_and_barrier(self, tick_clock, wait_clock):
            gc = list(tick_clock.global_clock)
            for i, v in enumerate(gc):
                if v:
                    single = [0] * len(gc)
                    single[i] = v
                    d = self.nc.sync.drain()
                    wait_clock.add_sem_waits(
                        d.ins, ScopedClock({None: bass_rust.VectorClock(single)})
                    )
            self.nc.all_engine_barrier()
            assert self.sems is not None
            popped = self.nc._tile_sem_poison_stack.pop()
            assert popped is self._sem_poison
            self.nc.clear_and_free_semaphores(list(self.sems.allocated().values()))
            self.nc.all_engine_barrier()

    return PatchedTC


def _split_excess_waits(nc):
    """Walrus here allows one sem wait per instruction; hoist extras onto
    same-engine NoOps inserted immediately before the instruction."""
    import bass_rust

    nid = 0
    for blk in nc.main_func.blocks:
        out = []
        for ins in blk.instructions:
            si = getattr(ins, "sync_info", None)
            waits = list(si.on_wait) if si is not None else []
            if len(waits) > 1:
                for w in waits[:-1]:
                    nid += 1
                    nop = bass_rust.InstNoOp(
                        name=f"I-waitsplit-{nid}", ins=[], outs=[])
                    nop.engine = ins.engine
                    nop.sync_info = bass_rust.SyncInfo(
                        on_wait=[w], on_update=[])
                    nc.register_instruction(nop, overwrite=True)
                    out.append(nop)
                si.on_wait = [waits[-1]]
                ins.sync_info = si
            out.append(ins)
        blk.instructions = out


def _ovl_pairs(ap2d, n, pair_stride=1):
    """AP reading, for each of n positions, the pair (v[l], v[l+pair_stride])
    as an innermost dim of 2 — input for a pairwise tensor_reduce."""
    import concourse.bass as bass

    o = ap2d.opt()
    return bass.AP(tensor=ap2d.tensor, offset=o.offset,
                   ap=[list(o.ap[0]), [1, n], [pair_stride, 2]])


def _build_program():
    import concourse.bass as bass
    import concourse.mybir as mybir

    PatchedTC = _make_patched_tc()

    f32 = mybir.dt.float32
    bf16 = mybir.dt.bfloat16
    u16 = mybir.dt.uint16
    Alu = mybir.AluOpType
    Act = mybir.ActivationFunctionType
    X = mybir.AxisListType.X

    nc = bass.Bass(target_bir_lowering=False, num_devices=NCORES)

    x_ext = nc.declare_dram_parameter("x", [C, L], bf16, isOutput=False)
    wb3t_ext = nc.declare_dram_parameter("wb3t", [C, C], bf16, isOutput=False)
    wa2t_ext = nc.declare_dram_parameter("wa2t", [C, C], bf16, isOutput=False)
    wdt_ext = nc.declare_dram_parameter("wdt", [C, C], bf16, isOutput=False)
    c2t4_ext = nc.declare_dram_parameter("c2t4", [C, C], bf16, isOutput=False)
    wcho_ext = nc.declare_dram_parameter("wcho", [C, C], f32, isOutput=False)
    wchi_ext = nc.declare_dram_parameter("wchi", [C, C], f32, isOutput=False)
    wm1t_ext = nc.declare_dram_parameter("wm1t", [C, 32], f32, isOutput=False)
    taps_ext = nc.declare_dram_parameter("taps", [C, 3], f32, isOutput=False)
    b1t_ext = nc.declare_dram_parameter("b1t", [C, 1], f32, isOutput=False)
    bout_ext = nc.declare_dram_parameter("bout", [C, 1], f32, isOutput=False)
    gb_ext = nc.declare_dram_parameter("gb", [C, 2], f32, isOutput=False)
    y_ext = nc.declare_dram_parameter("y", [C, L], bf16, isOutput=True)

    with PatchedTC(nc) as tc:
        with (
            tc.tile_pool(name="wp", bufs=1) as wp,
            tc.tile_pool(name="big", bufs=1) as big,
            tc.tile_pool(name="win", bufs=3) as win,
            tc.tile_pool(name="sm", bufs=1) as sm,
            tc.tile_pool(name="ed", bufs=2) as ed,
            tc.tile_pool(name="dump", bufs=2) as dump,
            tc.tile_pool(name="yps", bufs=5, space="PSUM") as yps,
            tc.tile_pool(name="hps", bufs=2, space="PSUM") as hps,
            tc.tile_pool(name="sps", bufs=1, space="PSUM") as sps,
            tc.tile_pool(name="dram", bufs=1, space="DRAM") as dram,
        ):
            # ---- weights to SBUF ----
            wb3t = wp.tile([C, C], bf16)
            wa2t = wp.tile([C, C], bf16)
            wdt = wp.tile([C, C], bf16)
            c2t4 = wp.tile([C, C], bf16)
            wcho = wp.tile([C, C], f32)
            wchi = wp.tile([C, C], f32)
            wm1t = wp.tile([C, 32], f32)
            taps = wp.tile([C, 3], f32)
            b1t = wp.tile([C, 1], f32)
            bout = wp.tile([C, 1], f32)
            gb = wp.tile([C, 2], f32)
            ones_row = wp.tile([1, C], f32)
            nc.vector.memset(ones_row, 1.0)
            for t, e in [(wb3t, wb3t_ext), (wa2t, wa2t_ext), (wdt, wdt_ext),
                         (c2t4, c2t4_ext), (wcho, wcho_ext), (wchi, wchi_ext),
                         (wm1t, wm1t_ext), (taps, taps_ext), (b1t, b1t_ext),
                         (bout, bout_ext), (gb, gb_ext)]:
                nc.gpsimd.dma_start(out=t, in_=e[:])

            # ---- big SBUF arrays ----
            xbf = big.tile([C, L], bf16)     # x (bf16, cast on host)
            ypre = big.tile([C, L], bf16)    # pre-BN output
            h1sb = big.tile([C, NGRP * NCH], bf16)  # silu(h1) packed 4ch/grp

            gsums = sm.tile([C, NGRP], f32)
            ysum = sm.tile([C, NCHUNK], f32)
            ysq = sm.tile([C, NCHUNK], f32)

            # ---- load x (window 7 early: chunk 0's wrap matmul reads it),
            #      accumulate per-window row sums on ACT (idle early) ----
            for g in [0, 7, 1, 2, 3, 4, 5, 6]:
                lo, hi = g * NW, (g + 1) * NW
                nc.sync.dma_start(out=xbf[:, lo:hi], in_=x_ext[:, lo:hi])
                gdump = dump.tile([C, NW], bf16, tag="gs")
                nc.scalar.activation(gdump, xbf[:, lo:hi], Act.Copy,
                                     accum_out=gsums[:, g:g + 1])

            # ---- channel-branch small chain (needs all of x) ----
            gsum = sm.tile([C, 1], f32)
            nc.vector.tensor_reduce(gsum, gsums, X, Alu.add)
            ss_ps = sps.tile([1, 1], f32, tag="sp")
            nc.tensor.matmul(ss_ps, gsum, gsum, start=True, stop=True)
            ss = sm.tile([1, 1], f32)
            nc.vector.tensor_copy(ss, ss_ps)
            rn2 = sm.tile([1, 1], f32)
            nc.vector.reciprocal(rn2, ss)          # 1 / ||gsum||^2

            v_ps = sps.tile([C, 1], f32, tag="sp")
            nc.tensor.matmul(v_ps, wcho, gsum, start=True, stop=True)
            v_sb = sm.tile([C, 1], f32)
            nc.vector.tensor_copy(v_sb, v_ps)
            pqr = sm.tile([C, 3], f32)
            for j in range(3):
                nc.vector.tensor_tensor(pqr[:, j:j + 1], v_sb, taps[:, j:j + 1],
                                        Alu.mult)
            pqr2_ps = sps.tile([C, 3], f32, tag="sp")
            nc.tensor.matmul(pqr2_ps, wchi, pqr, start=True, stop=True)
            pqr2 = sm.tile([C, 3], f32)
            nc.vector.tensor_copy(pqr2, pqr2_ps)

            u_ps = sps.tile([1, 32], f32, tag="sp")
            nc.tensor.matmul(u_ps, gsum, wm1t, start=True, stop=True)
            u_sb = sm.tile([1, 32], f32)
            nc.vector.tensor_copy(u_sb, u_ps)
            u_sc = sm.tile([1, 32], f32)
            nc.vector.tensor_scalar(u_sc, u_sb, rn2[0:1, 0:1], None, Alu.mult)
            # broadcast [1,32] -> [C,32] with a K=1 ones matmul (no DRAM trip)
            ub_ps = sps.tile([C, 32], f32, tag="sp")
            nc.tensor.matmul(ub_ps, ones_row, u_sc, start=True, stop=True)
            u_bc = sm.tile([C, 32], f32)
            nc.vector.tensor_copy(u_bc, ub_ps)

            mqt = sm.tile([C, 32], bf16)
            mpt = sm.tile([C, 32], bf16)
            mrt = sm.tile([C, 32], bf16)
            for t, j in [(mpt, 0), (mqt, 1), (mrt, 2)]:
                nc.vector.tensor_scalar(t, u_bc, pqr2[:, j:j + 1], None,
                                        Alu.mult)

            # ---- streaming main loop: per 2048-window DVE arrays, then the
            #      4 chunks of that window on PE/ACT; h1 group k+1 produced
            #      while window k+1's DVE arrays are being built ----
            def h1_group(k):
                h1ps = hps.tile([C, NCH], f32)
                for wgt, shift in [(mqt, 0), (mpt, -1), (mrt, +1)]:
                    for j in range(4):
                        n = 4 * k + j
                        n0 = n * NCH
                        lo = n0 + shift
                        hi = n0 + NCH + shift
                        plo, phi = 0, NCH
                        if lo < 0:
                            plo, lo = 1, 0
                        if hi > L:
                            phi, hi = NCH - 1, L
                        nc.tensor.matmul(
                            h1ps[32 * j:32 * j + 32, plo:phi],
                            wgt[:, 0:32], xbf[:, lo:hi],
                            start=(shift == 0), stop=(shift == 1),
                            tile_position=(0, 32 * j))
                nc.scalar.activation(h1sb[:, k * NCH:(k + 1) * NCH], h1ps,
                                     Act.Silu, bias=b1t[:, 0:1])

            def window_arrays(g):
                """DVE window arrays: H, V (diff branch), sxh, sxv (spatial)."""
                G0 = g * NW
                dh = win.tile([C, NW + 1], bf16, tag="dh")
                H = win.tile([C, NW], bf16, tag="H")
                dv = win.tile([C, NW + 128], bf16, tag="dv")
                V = win.tile([C, NW], bf16, tag="V")
                sxh = win.tile([C, NW], bf16, tag="sxh")
                sxv = win.tile([C, NW], bf16, tag="sxv")

                # dh[j] = x[G0+j] - x[G0+j-1], j in [a, e)
                a = 1 if g == 0 else 0
                e = NW if g == NGRP - 1 else NW + 1
                nc.vector.tensor_tensor(dh[:, a:e],
                                        xbf[:, G0 + a:G0 + e],
                                        xbf[:, G0 + a - 1:G0 + e - 1],
                                        Alu.subtract)
                # zero the row-start cols (reflect: no cross-row diffs)
                dh2 = dh[:, 0:NW].rearrange("p (r c) -> p r c", c=Wimg)
                nc.vector.memset(dh2[:, :, 0:1], 0.0)
                nc.vector.memset(dh[:, NW:NW + 1], 0.0)
                # H[j] = |dh[j]| + |dh[j+1]| (overlapping-pair abs reduce)
                with nc.allow_low_precision(reason="bf16 pair sum"):
                    nc.vector.tensor_reduce(H, _ovl_pairs(dh[:, 0:NW + 1], NW),
                                            X, Alu.add,
                                            apply_absolute_value=True)
                # edge fix: col0 += |dh[row,1]| ; col127 += |dh[row,127]|
                H2 = H.rearrange("p (r c) -> p r c", c=Wimg)
                dh2u = dh2.bitcast(u16)
                for src, dst in ((1, 0), (Wimg - 1, Wimg - 1)):
                    tmp = ed.tile([C, NROW], bf16, tag="he")
                    tmp3 = tmp.rearrange("p (r c) -> p r c", c=1)
                    tmpu = tmp3.bitcast(u16)
                    nc.vector.tensor_scalar(tmpu, dh2u[:, :, src:src + 1],
                                            0x7FFF, None, Alu.bitwise_and)
                    nc.vector.tensor_tensor(H2[:, :, dst:dst + 1],
                                            H2[:, :, dst:dst + 1], tmp3,
                                            Alu.add)

                # dv[j] = x[G0+j] - x[G0+j-128], j in [av, ev); |.| in place
                av = 128 if g == 0 else 0
                ev = NW if g == NGRP - 1 else NW + 128
                nc.vector.tensor_tensor(dv[:, av:ev], xbf[:, G0 + av:G0 + ev],
                                        xbf[:, G0 + av - 128:G0 + ev - 128],
                                        Alu.subtract)
                dvu = dv.bitcast(u16)
                nc.vector.tensor_scalar(dvu[:, av:ev], dvu[:, av:ev], 0x7FFF,
                                        None, Alu.bitwise_and)
                if g == 0:
                    nc.vector.memset(dv[:, 0:128], 0.0)   # row 0: no up-diff
                if g == NGRP - 1:
                    # last row reflect: pair partner := own value -> 2|dv|
                    nc.vector.tensor_copy(dv[:, NW:NW + 128],
                                          dv[:, NW - 128:NW])
                # V[j] = |dv[j]| + |dv[j+128]|
                nc.vector.tensor_tensor(V, dv[:, 0:NW], dv[:, 128:NW + 128],
                                        Alu.add)
                if g == 0:
                    # row 0 reflect: V = 2*|dv[j+128]|
                    nc.vector.tensor_tensor(V[:, 0:128], V[:, 0:128],
                                            dv[:, 128:256], Alu.add)

                # sxh[j] = x[l-1] + x[l+1] (flat scan, zero-pad ends)
                ha = 1 if g == 0 else 0
                hb = NW - 1 if g == NGRP - 1 else NW
                nc.vector.tensor_tensor(sxh[:, ha:hb],
                                        xbf[:, G0 + ha - 1:G0 + hb - 1],
                                        xbf[:, G0 + ha + 1:G0 + hb + 1],
                                        Alu.add)
                if g == 0:
                    nc.vector.tensor_copy(sxh[:, 0:1], xbf[:, 1:2])
                if g == NGRP - 1:
                    nc.vector.tensor_copy(sxh[:, NW - 1:NW],
                                          xbf[:, L - 2:L - 1])
                # sxv[j] = x[l-128] + x[l+128] (flat scan, zero-pad ends)
                va = 128 if g == 0 else 0
                vb = NW - 128 if g == NGRP - 1 else NW
                nc.vector.tensor_tensor(sxv[:, va:vb],
                                        xbf[:, G0 + va - 128:G0 + vb - 128],
                                        xbf[:, G0 + va + 128:G0 + vb + 128],
                                        Alu.add)
                if g == 0:
                    nc.vector.tensor_copy(sxv[:, 0:128], xbf[:, 128:256])
                if g == NGRP - 1:
                    nc.vector.tensor_copy(sxv[:, NW - 128:NW],
                                          xbf[:, L - 256:L - 128])
                return H, V, sxh, sxv

            def chunk(n, H, V, sxh, sxv):
                k, j = n // 4, n % 4
                n0 = n * NCH
                off = j * NCH
                ps = yps.tile([C, NCH], f32)
                nc.tensor.matmul(ps, wb3t, xbf[:, n0:n0 + NCH],
                                 start=True, stop=False)
                nc.tensor.matmul(ps, wa2t, sxh[:, off:off + NCH],
                                 start=False, stop=False)
                nc.tensor.matmul(ps, wa2t, sxv[:, off:off + NCH],
                                 start=False, stop=False)
                if n == 0:
                    # col-scan wrap: l=j gets x[(h-1)w + j - 1]
                    nc.tensor.matmul(ps[:, 1:128], wa2t,
                                     xbf[:, L - Wimg:L - 1],
                                     start=False, stop=False)
                if n == NCHUNK - 1:
                    # col-scan wrap: l=(h-1)w+j gets x[j+1]
                    nc.tensor.matmul(ps[:, NCH - 128:NCH - 1], wa2t,
                                     xbf[:, 1:128], start=False, stop=False)
                nc.tensor.matmul(ps, wdt, H[:, off:off + NCH],
                                 start=False, stop=False)
                nc.tensor.matmul(ps, wdt, V[:, off:off + NCH],
                                 start=False, stop=False)
                # channel contribution (row-tiled, K=32)
                nc.tensor.matmul(ps, c2t4[32 * j:32 * j + 32, :],
                                 h1sb[32 * j:32 * j + 32, k * NCH:(k + 1) * NCH],
                                 start=False, stop=True,
                                 tile_position=(32 * j, 0))
                # evacuate + per-chunk channel sums
                nc.scalar.activation(ypre[:, n0:n0 + NCH], ps, Act.Identity,
                                     bias=bout[:, 0:1],
                                     accum_out=ysum[:, n:n + 1])
                # sum of squares straight off PSUM (Square ignores the bias;
                # corrected below via cross-term algebra)
                dmp = dump.tile([C, NCH], bf16, tag="sq")
                nc.scalar.activation(dmp, ps, Act.Square,
                                     accum_out=ysq[:, n:n + 1])

            h1_group(0)
            win_arrays = window_arrays(0)
            for k in range(NGRP):
                nxt = None
                if k + 1 < NGRP:
                    h1_group(k + 1)
                    nxt = window_arrays(k + 1)
                for j in range(4):
                    chunk(4 * k + j, *win_arrays)
                win_arrays = nxt

            # ---- global BN stats via AllReduce ----
            # ysq tracked z = y - bout (PSUM, pre-bias):
            #   sum(y^2) = sum(z^2) + 2*bout*sum(y) - L*bout^2
            stats = sm.tile([C, 2], f32)
            nc.vector.tensor_reduce(stats[:, 0:1], ysum, X, Alu.add)
            nc.vector.tensor_reduce(stats[:, 1:2], ysq, X, Alu.add)
            cb = sm.tile([C, 1], f32)
            nc.vector.tensor_tensor(cb, bout, stats[:, 0:1], Alu.mult)
            nc.vector.scalar_tensor_tensor(stats[:, 1:2], cb, 2.0,
                                           stats[:, 1:2], Alu.mult, Alu.add)
            bsq = sm.tile([C, 1], f32)
            nc.vector.tensor_tensor(bsq, bout, bout, Alu.mult)
            nc.vector.scalar_tensor_tensor(stats[:, 1:2], bsq, -float(L),
                                           stats[:, 1:2], Alu.mult, Alu.add)
            # prefetch the sqrt ACT table while the collective runs
            sqpre = sm.tile([C, 1], f32)
            nc.scalar.activation(sqpre, stats[:, 1:2], Act.Sqrt)
            cc_in = dram.tile([C, 2], f32)
            cc_out = dram.tile([C, 2], f32)
            nc.gpsimd.dma_start(out=cc_in[:], in_=stats)
            nc.gpsimd.collective_compute(
                "AllReduce", Alu.add,
                replica_groups=[list(range(NCORES))],
                ins=[cc_in.opt()], outs=[cc_out.opt()])
            statsr = sm.tile([C, 2], f32)
            nc.gpsimd.dma_start(out=statsr, in_=cc_out[:])

            mean = sm.tile([C, 1], f32)
            ex2 = sm.tile([C, 1], f32)
            nc.vector.tensor_scalar(mean, statsr[:, 0:1], 1.0 / NTOT, None,
                                    Alu.mult)
            nc.vector.tensor_scalar(ex2, statsr[:, 1:2], 1.0 / NTOT, None,
                                    Alu.mult)
            m2 = sm.tile([C, 1], f32)
            nc.vector.tensor_tensor(m2, mean, mean, Alu.mult)
            varep = sm.tile([C, 1], f32)
            nc.vector.tensor_tensor(varep, ex2, m2, Alu.subtract)
            nc.vector.tensor_scalar(varep, varep, EPS_BN, None, Alu.add)
            inv = sm.tile([C, 1], f32)
            nc.vector.reciprocal(inv, varep)
            rstd = sm.tile([C, 1], f32)
            nc.scalar.activation(rstd, inv, Act.Sqrt)
            s_sc = sm.tile([C, 1], f32)
            nc.vector.tensor_tensor(s_sc, rstd, gb[:, 0:1], Alu.mult)
            ms = sm.tile([C, 1], f32)
            nc.vector.tensor_tensor(ms, mean, s_sc, Alu.mult)
            t_sc = sm.tile([C, 1], f32)
            nc.vector.tensor_tensor(t_sc, gb[:, 1:2], ms, Alu.subtract)

            # ---- apply BN (DVE 2x, bf16 out), write out ----
            for g in range(NGRP):
                lo, hi = g * NW, (g + 1) * NW
                ow = dump.tile([C, NW], bf16, tag="ow")
                nc.vector.tensor_scalar(ow, ypre[:, lo:hi],
                                        s_sc[:, 0:1], t_sc[:, 0:1],
                                        Alu.mult, Alu.add)
                nc.sync.dma_start(out=y_ext[:, lo:hi], in_=ow)

    _split_excess_waits(nc)
    return nc


def _fold_weights(inputs):
    f = np.float32
    W_in = inputs["w_spatial_in"].astype(np.float64)
    W_out = inputs["w_spatial_out"].astype(np.float64)
    dw_sp = inputs["w_dw_spatial"][:, 0, :].astype(np.float64)
    W_proj = inputs["w_out_proj"].astype(np.float64)
    W_mlp2 = inputs["w_mlp2"].astype(np.float64)
    dwt = float(inputs["diff_weight"])

    a_sym = dw_sp[:, 0] + dw_sp[:, 2]
    w1 = dw_sp[:, 1]
    A2 = 0.25 * W_proj @ (W_out * a_sym[None, :]) @ W_in
    B3 = W_proj @ (W_out * w1[None, :]) @ W_in + W_proj
    W_d = 0.25 * dwt * W_proj
    C2 = W_proj @ W_mlp2                     # [c, 32]
    bias_out = W_proj @ inputs["b_mlp2"].astype(np.float64)

    bf = ml_dtypes.bfloat16
    return {
        "wb3t": np.ascontiguousarray(B3.T.astype(bf)),
        "wa2t": np.ascontiguousarray(A2.T.astype(bf)),
        "wdt": np.ascontiguousarray(W_d.T.astype(bf)),
        "c2t4": np.ascontiguousarray(np.tile(C2.T.astype(bf), (4, 1))),
        "wcho": np.ascontiguousarray(inputs["w_ch_out"].astype(f)),
        "wchi": np.ascontiguousarray(inputs["w_ch_in"].astype(f)),
        "wm1t": np.ascontiguousarray(inputs["w_mlp1"].T.astype(f)),
        "taps": np.ascontiguousarray(inputs["w_ch_dw"][:, 0, :].astype(f)),
        "b1t": np.ascontiguousarray(
            np.tile(inputs["b_mlp1"].astype(f), 4)[:, None]),
        "bout": np.ascontiguousarray(bias_out.astype(f)[:, None]),
        "gb": np.ascontiguousarray(
            np.stack([inputs["bn_gamma"], inputs["bn_beta"]], 1).astype(f)),
    }


def prepare_in_maps(inputs):
    wmap = _fold_weights(inputs)
    x = np.asarray(inputs["x"]).astype(np.float32)  # [B, C, H, W]
    in_maps = []
    for b in range(NCORES):
        m = dict(wmap)
        m["x"] = np.ascontiguousarray(
            x[b].reshape(C, L).astype(ml_dtypes.bfloat16))
        in_maps.append(m)
    return in_maps


def kernel(**inputs):
    from concourse.bass_utils import run_bass_kernel_spmd

    inputs = {k: np.asarray(v) for k, v in inputs.items()}
    if "nc" not in _CACHE:
        _CACHE["nc"] = _build_program()
    nc = _CACHE["nc"]

    in_maps = prepare_in_maps(inputs)
    res = run_bass_kernel_spmd(nc, in_maps, list(range(NCORES)))
    out = np.stack([np.asarray(res.results[b]["y"]).astype(np.float32)
                    .reshape(C, Himg, Wimg) for b in range(NCORES)])
    return out


# revision 6
# speedup vs baseline: 1.1255x; 1.0284x over previous
"""OCS fused kernel for Trainium2, data-parallel over batch across 8 cores.

Algebraic restructuring (verified vs reference to ~1e-6 in fp64):

Spatial branch (4 scan orders, shared weights) collapses to a symmetric
5-point stencil with scan-order wrap rules, and the two 1x1 convs fold
through it:  W_proj @ y_sp = A2 @ (4-neighbor sum of x) + (B3 - W_proj) @ x.
The 4-neighbor sums are not materialized: A2 is applied as two fp8
DoubleRow matmuls, each fusing a +-shift pair of x (second K-half read via
a strided AP view), with the A2 magnitude rescaled 2^4 into fp8 range and
x pre-scaled 2^-4 on host (A2 term is ~0.7% of y, fp8 error is negligible
there). Col-scan wraps are two extra small bf16 matmuls.
Channel branch: m = g g^T is rank-1, so the whole conv pipeline collapses
into three [32,128] matmuls on shifted x (weights MP/MQ/MR = u (x) P/Q/R
built on-device from g = sum of x), a silu, and one [128,32] matmul.
Diff branch (large contributor -> bf16): |dx| per axis once (H-axis abs on
DVE, V-axis abs offloaded to GpSimd), pair-sums as shifted adds, W_proj
folds in as W_d @ H + W_d @ V.
BatchNorm: per-core partial (sum, sumsq) -> 1KB AllReduce (warmed up by a
dummy collective at kernel start so the mesh-algo load is off the critical
path) -> affine applied at DVE 2x with bf16 output (host upcasts to f32).
"""

import numpy as np
import ml_dtypes

B, C, Himg, Wimg = 8, 128, 128, 128
L = Himg * Wimg            # 16384
NCORES = 8
NCH = 512                  # matmul chunk columns
NCHUNK = L // NCH          # 32
NPAIR = NCHUNK // 2        # chunk pairs -> 2-bank psum tiles
NW = 2048                  # elementwise window columns (4 chunks)
NGRP = L // NW             # 8
NROW = NW // Wimg          # image rows per window (16)
EPS_BN = 1e-5
NTOT = float(B * L)        # batchnorm population per channel
XSC = 2.0 ** -4            # host prescale of the fp8 x copy
ASC = 2.0 ** 4             # fp8 A2 weight upscale (cancels XSC)

_CACHE = {}


def _make_patched_tc():
    """TileContext whose exit drain splits sem waits one-per-Drain.

    The walrus build in this container rejects Drain instructions carrying
    more than one sem wait ("Too many sync wait commands"). Stock
    TileContext attaches the whole global vector clock to a single tail
    Drain; emit one Drain per outstanding proc instead.
    """
    import bass_rust
    import concourse.tile as tile
    from concourse.vector_clock import ScopedClock

    class PatchedTC(tile.TileContext):
        def _drain_and_barrier(self, tick_clock, wait_clock):
            gc = list(tick_clock.global_clock)
            for i, v in enumerate(gc):
                if v:
                    single = [0] * len(gc)
                    single[i] = v
                    d = self.nc.sync.drain()
                    wait_clock.add_sem_waits(
                        d.ins, ScopedClock({None: bass_rust.VectorClock(single)})
                    )
            self.nc.all_engine_barrier()
            assert self.sems is not None
            popped = self.nc._tile_sem_poison_stack.pop()
            assert popped is self._sem_poison
            self.nc.clear_and_free_semaphores(list(self.sems.allocated().values()))
            self.nc.all_engine_barrier()

    return PatchedTC


def _split_excess_waits(nc):
    """Walrus here allows one sem wait per instruction; hoist extras onto
    same-engine NoOps inserted immediately before the instruction."""
    import bass_rust

    nid = 0
    for blk in nc.main_func.blocks:
        out = []
        for ins in blk.instructions:
            si = getattr(ins, "sync_info", None)
            waits = list(si.on_wait) if si is not None else []
            if len(waits) > 1:
                for w in waits[:-1]:
                    nid += 1
                    nop = bass_rust.InstNoOp(
                        name=f"I-waitsplit-{nid}", ins=[], outs=[])
                    nop.engine = ins.engine
                    nop.sync_info = bass_rust.SyncInfo(
                        on_wait=[w], on_update=[])
                    nc.register_instruction(nop, overwrite=True)
                    out.append(nop)
                si.on_wait = [waits[-1]]
                ins.sync_info = si
            out.append(ins)
        blk.instructions = out


def _build_program():
    import concourse.bass as bass
    import concourse.mybir as mybir

    PatchedTC = _make_patched_tc()

    f32 = mybir.dt.float32
    bf16 = mybir.dt.bfloat16
    fp8 = mybir.dt.float8e4
    u16 = mybir.dt.uint16
    Alu = mybir.AluOpType
    Act = mybir.ActivationFunctionType
    X = mybir.AxisListType.X
    DR = mybir.MatmulPerfMode.DoubleRow

    nc = bass.Bass(target_bir_lowering=False, num_devices=NCORES)

    x_ext = nc.declare_dram_parameter("x", [C, L], bf16, isOutput=False)
    x8_ext = nc.declare_dram_parameter("x8", [C, L], fp8, isOutput=False)
    wb3t_ext = nc.declare_dram_parameter("wb3t", [C, C], bf16, isOutput=False)
    a2d_ext = nc.declare_dram_parameter("a2d", [C, 2 * C], fp8, isOutput=False)
    wa2t_ext = nc.declare_dram_parameter("wa2t", [C, C], bf16, isOutput=False)
    wdt_ext = nc.declare_dram_parameter("wdt", [C, C], bf16, isOutput=False)
    c2t4_ext = nc.declare_dram_parameter("c2t4", [C, C], bf16, isOutput=False)
    wcho_ext = nc.declare_dram_parameter("wcho", [C, C], f32, isOutput=False)
    wchi_ext = nc.declare_dram_parameter("wchi", [C, C], f32, isOutput=False)
    wm1t_ext = nc.declare_dram_parameter("wm1t", [C, 32], f32, isOutput=False)
    taps_ext = nc.declare_dram_parameter("taps", [C, 3], f32, isOutput=False)
    b1t_ext = nc.declare_dram_parameter("b1t", [C, 1], f32, isOutput=False)
    bout_ext = nc.declare_dram_parameter("bout", [C, 1], f32, isOutput=False)
    gb_ext = nc.declare_dram_parameter("gb", [C, 2], f32, isOutput=False)
    y_ext = nc.declare_dram_parameter("y", [C, L], bf16, isOutput=True)

    with PatchedTC(nc) as tc:
        with (
            tc.tile_pool(name="wp", bufs=1) as wp,
            tc.tile_pool(name="big", bufs=1) as big,
            tc.tile_pool(name="win", bufs=3) as win,
            tc.tile_pool(name="sm", bufs=1) as sm,
            tc.tile_pool(name="dump", bufs=2) as dump,
            tc.tile_pool(name="ow", bufs=4) as owp,
            tc.tile_pool(name="yps", bufs=3, space="PSUM") as yps,
            tc.tile_pool(name="hps", bufs=1, space="PSUM") as hps,
            tc.tile_pool(name="sps", bufs=1, space="PSUM") as sps,
            tc.tile_pool(name="dram", bufs=1, space="DRAM") as dram,
        ):
            # ---- weights to SBUF (gpsimd queue; x on scalar/sync queues) ----
            wb3t = wp.tile([C, C], bf16)
            a2d = wp.tile([C, 2 * C], fp8)
            wa2t = wp.tile([C, C], bf16)
            wdt = wp.tile([C, C], bf16)
            c2t4 = wp.tile([C, C], bf16)
            wcho = wp.tile([C, C], f32)
            wchi = wp.tile([C, C], f32)
            wm1t = wp.tile([C, 32], f32)
            taps = wp.tile([C, 3], f32)
            b1t = wp.tile([C, 1], f32)
            bout = wp.tile([C, 1], f32)
            gb = wp.tile([C, 2], f32)
            ones_row = wp.tile([1, C], f32)
            nc.vector.memset(ones_row, 1.0)
            for t, e in [(wb3t, wb3t_ext), (a2d, a2d_ext), (wa2t, wa2t_ext),
                         (wdt, wdt_ext), (c2t4, c2t4_ext), (wcho, wcho_ext),
                         (wchi, wchi_ext), (wm1t, wm1t_ext), (taps, taps_ext),
                         (b1t, b1t_ext), (bout, bout_ext), (gb, gb_ext)]:
                nc.gpsimd.dma_start(out=t, in_=e[:])

            # warmup collective: loads the CC mesh algo while compute runs,
            # so the real stats AllReduce skips the ~11us startup
            ccw_in = dram.tile([C, 2], f32)
            ccw_out = dram.tile([C, 2], f32)
            nc.gpsimd.dma_start(out=ccw_in[:], in_=gb)
            nc.gpsimd.collective_compute(
                "AllReduce", Alu.add,
                replica_groups=[list(range(NCORES))],
                ins=[ccw_in.opt()], outs=[ccw_out.opt()])

            # ---- big SBUF arrays ----
            xbf = big.tile([C, L], bf16)     # x (bf16, cast on host)
            x8 = big.tile([C, L], fp8)       # x * 2^-4 (fp8, cast on host)
            ypre = big.tile([C, L], bf16)    # pre-BN output
            h1sb = big.tile([C, NGRP * NCH], bf16)  # silu(h1) packed 4ch/grp

            gsums = sm.tile([C, NGRP], f32)
            ysum = sm.tile([C, NPAIR], f32)
            ysq = sm.tile([C, NPAIR], f32)

            # ---- load x (window 7 early: chunk 0's wrap matmul reads it),
            #      accumulate per-window row sums on ACT (idle early) ----
            nc.sync.dma_start(out=x8, in_=x8_ext[:])
            for g in [0, 7, 1, 2, 3, 4, 5, 6]:
                lo, hi = g * NW, (g + 1) * NW
                nc.scalar.dma_start(out=xbf[:, lo:hi], in_=x_ext[:, lo:hi])
            for g in [0, 7, 1, 2, 3, 4, 5, 6]:
                lo, hi = g * NW, (g + 1) * NW
                gdump = dump.tile([C, NW], bf16, tag="gs")
                nc.scalar.activation(gdump, xbf[:, lo:hi], Act.Copy,
                                     accum_out=gsums[:, g:g + 1])

            # ---- channel-branch small chain (needs all of x) ----
            gsum = sm.tile([C, 1], f32)
            nc.vector.tensor_reduce(gsum, gsums, X, Alu.add)
            ss_ps = sps.tile([1, 1], f32, tag="sp")
            nc.tensor.matmul(ss_ps, gsum, gsum, start=True, stop=True)
            ss = sm.tile([1, 1], f32)
            nc.vector.tensor_copy(ss, ss_ps)
            rn2 = sm.tile([1, 1], f32)
            nc.vector.reciprocal(rn2, ss)          # 1 / ||gsum||^2

            v_ps = sps.tile([C, 1], f32, tag="sp")
            nc.tensor.matmul(v_ps, wcho, gsum, start=True, stop=True)
            v_sb = sm.tile([C, 1], f32)
            nc.vector.tensor_copy(v_sb, v_ps)
            pqr = sm.tile([C, 3], f32)
            for j in range(3):
                nc.vector.tensor_tensor(pqr[:, j:j + 1], v_sb, taps[:, j:j + 1],
                                        Alu.mult)
            pqr2_ps = sps.tile([C, 3], f32, tag="sp")
            nc.tensor.matmul(pqr2_ps, wchi, pqr, start=True, stop=True)
            pqr2 = sm.tile([C, 3], f32)
            nc.vector.tensor_copy(pqr2, pqr2_ps)

            u_ps = sps.tile([1, 32], f32, tag="sp")
            nc.tensor.matmul(u_ps, gsum, wm1t, start=True, stop=True)
            u_sb = sm.tile([1, 32], f32)
            nc.vector.tensor_copy(u_sb, u_ps)
            u_sc = sm.tile([1, 32], f32)
            nc.vector.tensor_scalar(u_sc, u_sb, rn2[0:1, 0:1], None, Alu.mult)
            # broadcast [1,32] -> [C,32] with a K=1 ones matmul (no DRAM trip)
            ub_ps = sps.tile([C, 32], f32, tag="sp")
            nc.tensor.matmul(ub_ps, ones_row, u_sc, start=True, stop=True)
            u_bc = sm.tile([C, 32], f32)
            nc.vector.tensor_copy(u_bc, ub_ps)

            mqt = sm.tile([C, 32], bf16)
            mpt = sm.tile([C, 32], bf16)
            mrt = sm.tile([C, 32], bf16)
            for t, j in [(mpt, 0), (mqt, 1), (mrt, 2)]:
                nc.vector.tensor_scalar(t, u_bc, pqr2[:, j:j + 1], None,
                                        Alu.mult)

            # ---- streaming main loop ----
            def h1_group(k):
                h1ps = hps.tile([C, NCH], f32)
                for wgt, shift in [(mqt, 0), (mpt, -1), (mrt, +1)]:
                    for j in range(4):
                        n = 4 * k + j
                        n0 = n * NCH
                        lo = n0 + shift
                        hi = n0 + NCH + shift
                        plo, phi = 0, NCH
                        if lo < 0:
                            plo, lo = 1, 0
                        if hi > L:
                            phi, hi = NCH - 1, L
                        nc.tensor.matmul(
                            h1ps[32 * j:32 * j + 32, plo:phi],
                            wgt[:, 0:32], xbf[:, lo:hi],
                            start=(shift == 0), stop=(shift == 1),
                            tile_position=(0, 32 * j))
                nc.scalar.activation(h1sb[:, k * NCH:(k + 1) * NCH], h1ps,
                                     Act.Silu, bias=b1t[:, 0:1])

            def dr_pair(base, istride, n):
                """fp8 ifmap AP reading, for each of n cols j, the K-half pair
                (x8[base+j], x8[base+istride+j]) for a DoubleRow matmul."""
                anchor = x8[:, base:base + 1]
                o = anchor.opt()
                return bass.AP(tensor=anchor.tensor, offset=o.offset,
                               ap=[list(o.ap[0]), [istride, 2], [1, n]])

            a2w = a2d.rearrange("p (i m) -> p i m", i=2)
            a2s = a2d[:, 0:C]   # single (non-DR) fp8 A2 view

            def window_arrays(g):
                """Window arrays: H, V of the diff branch (V-axis abs on
                the otherwise-idle GpSimd engine)."""
                G0 = g * NW
                dh = win.tile([C, NW + 1], bf16, tag="dh")
                H = win.tile([C, NW], bf16, tag="H")
                dv = win.tile([C, NW + 128], bf16, tag="dv")
                V = win.tile([C, NW], bf16, tag="V")

                # dh[j] = x[G0+j] - x[G0+j-1], j in [a, e); |.| in place
                a = 1 if g == 0 else 0
                e = NW if g == NGRP - 1 else NW + 1
                nc.vector.tensor_tensor(dh[:, a:e],
                                        xbf[:, G0 + a:G0 + e],
                                        xbf[:, G0 + a - 1:G0 + e - 1],
                                        Alu.subtract)
                dh2 = dh[:, 0:NW].rearrange("p (r c) -> p r c", c=Wimg)
                nc.vector.memset(dh2[:, :, 0:1], 0.0)   # no cross-row diffs
                nc.vector.memset(dh[:, NW:NW + 1], 0.0)
                dhu = dh.bitcast(u16)
                nc.vector.tensor_scalar(dhu[:, a:e], dhu[:, a:e], 0x7FFF,
                                        None, Alu.bitwise_and)
                # H[j] = |dh[j]| + |dh[j+1]|
                nc.vector.tensor_tensor(H, dh[:, 0:NW], dh[:, 1:NW + 1],
                                        Alu.add)
                # edge fix: col0 += |dh[row,1]| ; col127 += |dh[row,127]|
                H2 = H.rearrange("p (r c) -> p r c", c=Wimg)
                nc.vector.tensor_tensor(H2[:, :, 0:1], H2[:, :, 0:1],
                                        dh2[:, :, 1:2], Alu.add)
                nc.vector.tensor_tensor(H2[:, :, Wimg - 1:Wimg],
                                        H2[:, :, Wimg - 1:Wimg],
                                        dh2[:, :, Wimg - 1:Wimg], Alu.add)

                # dv[j] = x[G0+j] - x[G0+j-128]; abs on GpSimd
                av = 128 if g == 0 else 0
                ev = NW if g == NGRP - 1 else NW + 128
                nc.vector.tensor_tensor(dv[:, av:ev], xbf[:, G0 + av:G0 + ev],
                                        xbf[:, G0 + av - 128:G0 + ev - 128],
                                        Alu.subtract)
                dvu = dv.bitcast(u16)
                nc.vector.tensor_scalar(dvu[:, av:ev], dvu[:, av:ev], 0x7FFF,
                                        None, Alu.bitwise_and)
                if g == 0:
                    nc.vector.memset(dv[:, 0:128], 0.0)   # row 0: no up-diff
                if g == NGRP - 1:
                    # last row reflect: pair partner := own value -> 2|dv|
                    nc.vector.tensor_copy(dv[:, NW:NW + 128],
                                          dv[:, NW - 128:NW])
                # V[j] = |dv[j]| + |dv[j+128]| (on the otherwise-idle GpSimd)
                nc.gpsimd.tensor_tensor(V, dv[:, 0:NW], dv[:, 128:NW + 128],
                                        Alu.add)
                if g == 0:
                    # row 0 reflect: V = 2*|dv[j+128]|
                    nc.vector.tensor_tensor(V[:, 0:128], V[:, 0:128],
                                            dv[:, 128:256], Alu.add)
                return H, V

            def half_chunk(ps, q, n, H, V):
                """All matmuls for chunk n into psum cols [q, q+512)."""
                n0 = n * NCH
                off = (n % 4) * NCH
                j = n % 4
                pso = ps[:, q:q + NCH]
                nc.tensor.matmul(pso, wb3t, xbf[:, n0:n0 + NCH],
                                 start=True, stop=False)
                # A2 @ (x[l-1]+x[l+1]) as one fp8 DoubleRow matmul
                plo = 1 if n == 0 else 0
                phi = NCH - 1 if n == NCHUNK - 1 else NCH
                nc.tensor.matmul(ps[:, q + plo:q + phi], a2w,
                                 dr_pair(n0 + plo - 1, 2, phi - plo),
                                 start=False, stop=False, perf_mode=DR)
                if n == 0:      # l=0 keeps only the right neighbor
                    nc.tensor.matmul(ps[:, q:q + 1], a2s, x8[:, 1:2],
                                     start=False, stop=False)
                if n == NCHUNK - 1:   # l=L-1 keeps only the left neighbor
                    nc.tensor.matmul(ps[:, q + NCH - 1:q + NCH], a2s,
                                     x8[:, L - 2:L - 1],
                                     start=False, stop=False)
                # A2 @ (x[l-128]+x[l+128]) as one fp8 DoubleRow matmul
                vlo = 128 if n == 0 else 0
                vhi = NCH - 128 if n == NCHUNK - 1 else NCH
                nc.tensor.matmul(ps[:, q + vlo:q + vhi], a2w,
                                 dr_pair(n0 + vlo - 128, 256, vhi - vlo),
                                 start=False, stop=False, perf_mode=DR)
                if n == 0:      # first image row keeps only the down neighbor
                    nc.tensor.matmul(ps[:, q:q + 128], a2s, x8[:, 128:256],
                                     start=False, stop=False)
                if n == NCHUNK - 1:   # last image row keeps only up
                    nc.tensor.matmul(ps[:, q + NCH - 128:q + NCH], a2s,
                                     x8[:, L - 256:L - 128],
                                     start=False, stop=False)
                if n == 0:
                    # col-scan wrap: l=j gets x[(h-1)w + j - 1]
                    nc.tensor.matmul(ps[:, q + 1:q + 128], wa2t,
                                     xbf[:, L - Wimg:L - 1],
                                     start=False, stop=False)
                if n == NCHUNK - 1:
                    # col-scan wrap: l=(h-1)w+j gets x[j+1]
                    nc.tensor.matmul(ps[:, q + NCH - 128:q + NCH - 1], wa2t,
                                     xbf[:, 1:128], start=False, stop=False)
                # diff branch
                nc.tensor.matmul(pso, wdt, H[:, off:off + NCH],
                                 start=False, stop=False)
                nc.tensor.matmul(pso, wdt, V[:, off:off + NCH],
                                 start=False, stop=False)
                # channel contribution (row-tiled, K=32)
                nc.tensor.matmul(pso, c2t4[32 * j:32 * j + 32, :],
                                 h1sb[32 * j:32 * j + 32,
                                      (n // 4) * NCH:(n // 4 + 1) * NCH],
                                 start=False, stop=True,
                                 tile_position=(32 * j, 0))

            def chunk_pair(m, H, V):
                ps = yps.tile([C, 2 * NCH], f32)
                for h in range(2):
                    half_chunk(ps, h * NCH, 2 * m + h, H, V)
                n0 = 2 * m * NCH
                nc.scalar.activation(ypre[:, n0:n0 + 2 * NCH], ps,
                                     Act.Identity, bias=bout[:, 0:1],
                                     accum_out=ysum[:, m:m + 1])
                dmp = dump.tile([C, 2 * NCH], bf16, tag="sq")
                nc.scalar.activation(dmp, ps, Act.Square,
                                     accum_out=ysq[:, m:m + 1])

            h1_group(0)
            win_arrays = window_arrays(0)
            for k in range(NGRP):
                nxt = None
                if k + 1 < NGRP:
                    h1_group(k + 1)
                    nxt = window_arrays(k + 1)
                chunk_pair(2 * k, *win_arrays)
                chunk_pair(2 * k + 1, *win_arrays)
                win_arrays = nxt

            # ---- global BN stats via AllReduce ----
            # ysq tracked z = y - bout (PSUM, pre-bias):
            #   sum(y^2) = sum(z^2) + 2*bout*sum(y) - L*bout^2
            stats = sm.tile([C, 2], f32)
            nc.vector.tensor_reduce(stats[:, 0:1], ysum, X, Alu.add)
            nc.vector.tensor_reduce(stats[:, 1:2], ysq, X, Alu.add)
            cb = sm.tile([C, 1], f32)
            nc.vector.tensor_tensor(cb, bout, stats[:, 0:1], Alu.mult)
            nc.vector.scalar_tensor_tensor(stats[:, 1:2], cb, 2.0,
                                           stats[:, 1:2], Alu.mult, Alu.add)
            bsq = sm.tile([C, 1], f32)
            nc.vector.tensor_tensor(bsq, bout, bout, Alu.mult)
            nc.vector.scalar_tensor_tensor(stats[:, 1:2], bsq, -float(L),
                                           stats[:, 1:2], Alu.mult, Alu.add)
            # prefetch the sqrt ACT table while the collective runs
            sqpre = sm.tile([C, 1], f32)
            nc.scalar.activation(sqpre, stats[:, 1:2], Act.Sqrt)
            cc_in = dram.tile([C, 2], f32)
            cc_out = dram.tile([C, 2], f32)
            nc.gpsimd.dma_start(out=cc_in[:], in_=stats)
            nc.gpsimd.collective_compute(
                "AllReduce", Alu.add,
                replica_groups=[list(range(NCORES))],
                ins=[cc_in.opt()], outs=[cc_out.opt()])
            statsr = sm.tile([C, 2], f32)
            nc.gpsimd.dma_start(out=statsr, in_=cc_out[:])

            mean = sm.tile([C, 1], f32)
            ex2 = sm.tile([C, 1], f32)
            nc.vector.tensor_scalar(mean, statsr[:, 0:1], 1.0 / NTOT, None,
                                    Alu.mult)
            nc.vector.tensor_scalar(ex2, statsr[:, 1:2], 1.0 / NTOT, None,
                                    Alu.mult)
            m2 = sm.tile([C, 1], f32)
            nc.vector.tensor_tensor(m2, mean, mean, Alu.mult)
            varep = sm.tile([C, 1], f32)
            nc.vector.tensor_tensor(varep, ex2, m2, Alu.subtract)
            nc.vector.tensor_scalar(varep, varep, EPS_BN, None, Alu.add)
            inv = sm.tile([C, 1], f32)
            nc.vector.reciprocal(inv, varep)
            rstd = sm.tile([C, 1], f32)
            nc.scalar.activation(rstd, inv, Act.Sqrt)
            s_sc = sm.tile([C, 1], f32)
            nc.vector.tensor_tensor(s_sc, rstd, gb[:, 0:1], Alu.mult)
            ms = sm.tile([C, 1], f32)
            nc.vector.tensor_tensor(ms, mean, s_sc, Alu.mult)
            t_sc = sm.tile([C, 1], f32)
            nc.vector.tensor_tensor(t_sc, gb[:, 1:2], ms, Alu.subtract)

            # ---- apply BN (DVE 2x, bf16 out), write out on two queues ----
            for g in range(NGRP):
                lo, hi = g * NW, (g + 1) * NW
                ow = owp.tile([C, NW], bf16, tag="ow")
                nc.vector.tensor_scalar(ow, ypre[:, lo:hi],
                                        s_sc[:, 0:1], t_sc[:, 0:1],
                                        Alu.mult, Alu.add)
                eng = nc.sync if g % 2 == 0 else nc.scalar
                eng.dma_start(out=y_ext[:, lo:hi], in_=ow)

    _split_excess_waits(nc)
    return nc


def _fold_weights(inputs):
    f = np.float32
    W_in = inputs["w_spatial_in"].astype(np.float64)
    W_out = inputs["w_spatial_out"].astype(np.float64)
    dw_sp = inputs["w_dw_spatial"][:, 0, :].astype(np.float64)
    W_proj = inputs["w_out_proj"].astype(np.float64)
    W_mlp2 = inputs["w_mlp2"].astype(np.float64)
    dwt = float(inputs["diff_weight"])

    a_sym = dw_sp[:, 0] + dw_sp[:, 2]
    w1 = dw_sp[:, 1]
    A2 = 0.25 * W_proj @ (W_out * a_sym[None, :]) @ W_in
    B3 = W_proj @ (W_out * w1[None, :]) @ W_in + W_proj
    W_d = 0.25 * dwt * W_proj
    C2 = W_proj @ W_mlp2                     # [c, 32]
    bias_out = W_proj @ inputs["b_mlp2"].astype(np.float64)

    bf = ml_dtypes.bfloat16
    f8 = ml_dtypes.float8_e4m3
    a2t8 = np.ascontiguousarray((A2.T * ASC).astype(f8))
    return {
        "wb3t": np.ascontiguousarray(B3.T.astype(bf)),
        "a2d": np.ascontiguousarray(np.concatenate([a2t8, a2t8], axis=1)),
        "wa2t": np.ascontiguousarray(A2.T.astype(bf)),
        "wdt": np.ascontiguousarray(W_d.T.astype(bf)),
        "c2t4": np.ascontiguousarray(np.tile(C2.T.astype(bf), (4, 1))),
        "wcho": np.ascontiguousarray(inputs["w_ch_out"].astype(f)),
        "wchi": np.ascontiguousarray(inputs["w_ch_in"].astype(f)),
        "wm1t": np.ascontiguousarray(inputs["w_mlp1"].T.astype(f)),
        "taps": np.ascontiguousarray(inputs["w_ch_dw"][:, 0, :].astype(f)),
        "b1t": np.ascontiguousarray(
            np.tile(inputs["b_mlp1"].astype(f), 4)[:, None]),
        "bout": np.ascontiguousarray(bias_out.astype(f)[:, None]),
        "gb": np.ascontiguousarray(
            np.stack([inputs["bn_gamma"], inputs["bn_beta"]], 1).astype(f)),
    }


def prepare_in_maps(inputs):
    wmap = _fold_weights(inputs)
    x = np.asarray(inputs["x"]).astype(np.float32)  # [B, C, H, W]
    in_maps = []
    for b in range(NCORES):
        m = dict(wmap)
        xb = x[b].reshape(C, L)
        m["x"] = np.ascontiguousarray(xb.astype(ml_dtypes.bfloat16))
        m["x8"] = np.ascontiguousarray(
            (xb * XSC).astype(ml_dtypes.float8_e4m3))
        in_maps.append(m)
    return in_maps


def kernel(**inputs):
    from concourse.bass_utils import run_bass_kernel_spmd

    inputs = {k: np.asarray(v) for k, v in inputs.items()}
    if "nc" not in _CACHE:
        _CACHE["nc"] = _build_program()
    nc = _CACHE["nc"]

    in_maps = prepare_in_maps(inputs)
    res = run_bass_kernel_spmd(nc, in_maps, list(range(NCORES)))
    out = np.stack([np.asarray(res.results[b]["y"]).astype(np.float32)
                    .reshape(C, Himg, Wimg) for b in range(NCORES)])
    return out


# revision 12
# speedup vs baseline: 1.2423x; 1.1037x over previous
"""OCS fused kernel for Trainium2, data-parallel over batch across 8 cores.

Algebraic restructuring (verified vs reference to ~1e-6 in fp64):

Spatial branch (4 scan orders, shared weights) collapses to a symmetric
5-point stencil with scan-order wrap rules, and the two 1x1 convs fold
through it:  W_proj @ y_sp = A2 @ (4-neighbor sum of x) + (B3 - W_proj) @ x.
The 4-neighbor sums are not materialized: A2 is applied as two fp8
DoubleRow matmuls, each fusing a +-shift pair of x (second K-half read via
a strided AP view), with the A2 magnitude rescaled 2^4 into fp8 range and
x pre-scaled 2^-4 on host (A2 term is ~0.7% of y, fp8 error is negligible
there). Col-scan wraps are two extra small bf16 matmuls.
Channel branch: m = g g^T is rank-1, so the whole conv pipeline collapses
into three [32,128] matmuls on shifted x (weights MP/MQ/MR = u (x) P/Q/R
built on-device from g = sum of x), a silu, and one [128,32] matmul.
Diff branch (large contributor -> bf16): |dx| per axis once (H-axis abs on
DVE, V-axis abs offloaded to GpSimd), pair-sums as shifted adds, W_proj
folds in as W_d @ H + W_d @ V.
BatchNorm: per-core partial (sum, sumsq) -> 1KB AllReduce (warmed up by a
dummy collective at kernel start so the mesh-algo load is off the critical
path) -> affine applied at DVE 2x with bf16 output (host upcasts to f32).
"""

import numpy as np
import ml_dtypes

B, C, Himg, Wimg = 8, 128, 128, 128
L = Himg * Wimg            # 16384
NCORES = 8
NCH = 512                  # matmul chunk columns
NCHUNK = L // NCH          # 32
NPAIR = NCHUNK // 2        # chunk pairs -> 2-bank psum tiles
NW = 2048                  # elementwise window columns (4 chunks)
NGRP = L // NW             # 8
NROW = NW // Wimg          # image rows per window (16)
EPS_BN = 1e-5
NTOT = float(B * L)        # batchnorm population per channel
XSC = 2.0 ** -4            # host prescale of the fp8 x copy
ASC = 2.0 ** 4             # fp8 A2 weight upscale (cancels XSC)

_CACHE = {}


def _make_patched_tc():
    """TileContext whose exit drain splits sem waits one-per-Drain.

    The walrus build in this container rejects Drain instructions carrying
    more than one sem wait ("Too many sync wait commands"). Stock
    TileContext attaches the whole global vector clock to a single tail
    Drain; emit one Drain per outstanding proc instead.
    """
    import bass_rust
    import concourse.tile as tile
    from concourse.vector_clock import ScopedClock

    class PatchedTC(tile.TileContext):
        def _drain_and_barrier(self, tick_clock, wait_clock):
            gc = list(tick_clock.global_clock)
            for i, v in enumerate(gc):
                if v:
                    single = [0] * len(gc)
                    single[i] = v
                    d = self.nc.sync.drain()
                    wait_clock.add_sem_waits(
                        d.ins, ScopedClock({None: bass_rust.VectorClock(single)})
                    )
            self.nc.all_engine_barrier()
            assert self.sems is not None
            popped = self.nc._tile_sem_poison_stack.pop()
            assert popped is self._sem_poison
            self.nc.clear_and_free_semaphores(list(self.sems.allocated().values()))
            self.nc.all_engine_barrier()

    return PatchedTC


def _split_excess_waits(nc):
    """Walrus here allows one sem wait per instruction; hoist extras onto
    same-engine NoOps inserted immediately before the instruction."""
    import bass_rust

    nid = 0
    for blk in nc.main_func.blocks:
        out = []
        for ins in blk.instructions:
            si = getattr(ins, "sync_info", None)
            waits = list(si.on_wait) if si is not None else []
            if len(waits) > 1:
                for w in waits[:-1]:
                    nid += 1
                    nop = bass_rust.InstNoOp(
                        name=f"I-waitsplit-{nid}", ins=[], outs=[])
                    nop.engine = ins.engine
                    nop.sync_info = bass_rust.SyncInfo(
                        on_wait=[w], on_update=[])
                    nc.register_instruction(nop, overwrite=True)
                    out.append(nop)
                si.on_wait = [waits[-1]]
                ins.sync_info = si
            out.append(ins)
        blk.instructions = out


def _build_program():
    import concourse.bass as bass
    import concourse.mybir as mybir

    PatchedTC = _make_patched_tc()

    f32 = mybir.dt.float32
    bf16 = mybir.dt.bfloat16
    fp8 = mybir.dt.float8e4
    u16 = mybir.dt.uint16
    Alu = mybir.AluOpType
    Act = mybir.ActivationFunctionType
    X = mybir.AxisListType.X
    DR = mybir.MatmulPerfMode.DoubleRow

    nc = bass.Bass(target_bir_lowering=False, num_devices=NCORES)

    x_ext = nc.declare_dram_parameter("x", [C, L], bf16, isOutput=False)
    x8_ext = nc.declare_dram_parameter("x8", [C, L], fp8, isOutput=False)
    wb3t_ext = nc.declare_dram_parameter("wb3t", [C, C], bf16, isOutput=False)
    a2d_ext = nc.declare_dram_parameter("a2d", [C, 2 * C], fp8, isOutput=False)
    wa2t_ext = nc.declare_dram_parameter("wa2t", [C, C], bf16, isOutput=False)
    wdt_ext = nc.declare_dram_parameter("wdt", [C, C], bf16, isOutput=False)
    c2t4_ext = nc.declare_dram_parameter("c2t4", [C, C], bf16, isOutput=False)
    wcho_ext = nc.declare_dram_parameter("wcho", [C, C], f32, isOutput=False)
    wchi_ext = nc.declare_dram_parameter("wchi", [C, C], f32, isOutput=False)
    wm1t_ext = nc.declare_dram_parameter("wm1t", [C, 32], f32, isOutput=False)
    taps_ext = nc.declare_dram_parameter("taps", [C, 3], f32, isOutput=False)
    b1t_ext = nc.declare_dram_parameter("b1t", [C, 1], f32, isOutput=False)
    bout_ext = nc.declare_dram_parameter("bout", [C, 1], f32, isOutput=False)
    gb_ext = nc.declare_dram_parameter("gb", [C, 2], f32, isOutput=False)
    y_ext = nc.declare_dram_parameter("y", [C, L], bf16, isOutput=True)

    with PatchedTC(nc) as tc:
        with (
            tc.tile_pool(name="wp", bufs=1) as wp,
            tc.tile_pool(name="big", bufs=1) as big,
            tc.tile_pool(name="win", bufs=3) as win,
            tc.tile_pool(name="sm", bufs=1) as sm,
            tc.tile_pool(name="dump", bufs=2) as dump,
            tc.tile_pool(name="ow", bufs=4) as owp,
            tc.tile_pool(name="yps", bufs=3, space="PSUM") as yps,
            tc.tile_pool(name="hps", bufs=1, space="PSUM") as hps,
            tc.tile_pool(name="sps", bufs=1, space="PSUM") as sps,
            tc.tile_pool(name="dram", bufs=1, space="DRAM") as dram,
        ):
            # ---- weights to SBUF (gpsimd queue; x on scalar/sync queues) ----
            wb3t = wp.tile([C, C], bf16)
            a2d = wp.tile([C, 2 * C], fp8)
            wa2t = wp.tile([C, C], bf16)
            wdt = wp.tile([C, C], bf16)
            c2t4 = wp.tile([C, C], bf16)
            wcho = wp.tile([C, C], f32)
            wchi = wp.tile([C, C], f32)
            wm1t = wp.tile([C, 32], f32)
            taps = wp.tile([C, 3], f32)
            b1t = wp.tile([C, 1], f32)
            bout = wp.tile([C, 1], f32)
            gb = wp.tile([C, 2], f32)
            ones_row = wp.tile([1, C], f32)
            nc.vector.memset(ones_row, 1.0)
            for t, e in [(wb3t, wb3t_ext), (a2d, a2d_ext), (wa2t, wa2t_ext),
                         (wdt, wdt_ext), (c2t4, c2t4_ext), (wcho, wcho_ext),
                         (wchi, wchi_ext), (wm1t, wm1t_ext), (taps, taps_ext),
                         (b1t, b1t_ext), (bout, bout_ext), (gb, gb_ext)]:
                nc.gpsimd.dma_start(out=t, in_=e[:])

            # warmup collective: loads the CC mesh algo while compute runs,
            # so the real stats AllReduce skips the ~11us startup
            ccw_in = dram.tile([C, 2], f32)
            ccw_out = dram.tile([C, 2], f32)
            nc.gpsimd.dma_start(out=ccw_in[:], in_=gb)
            nc.gpsimd.collective_compute(
                "AllReduce", Alu.add,
                replica_groups=[list(range(NCORES))],
                ins=[ccw_in.opt()], outs=[ccw_out.opt()])

            # ---- big SBUF arrays ----
            xbf = big.tile([C, L], bf16)     # x (bf16, cast on host)
            x8 = big.tile([C, L], fp8)       # x * 2^-4 (fp8, cast on host)
            ypre = big.tile([C, L], bf16)    # pre-BN output
            h1sb = big.tile([C, NGRP * NCH], bf16)  # silu(h1) packed 4ch/grp

            gsums = sm.tile([C, NGRP], f32)
            ysum = sm.tile([C, NPAIR], f32)
            ysq = sm.tile([C, NPAIR], f32)

            # ---- load x (window 7 early: chunk 0's wrap matmul reads it),
            #      accumulate per-window row sums on ACT (idle early) ----
            nc.sync.dma_start(out=x8, in_=x8_ext[:])
            for g in [0, 7, 1, 2, 3, 4, 5, 6]:
                lo, hi = g * NW, (g + 1) * NW
                nc.scalar.dma_start(out=xbf[:, lo:hi], in_=x_ext[:, lo:hi])
            for g in [0, 7, 1, 2, 3, 4, 5, 6]:
                lo, hi = g * NW, (g + 1) * NW
                gdump = dump.tile([C, NW], bf16, tag="gs")
                nc.scalar.activation(gdump, xbf[:, lo:hi], Act.Copy,
                                     accum_out=gsums[:, g:g + 1])

            # ---- channel-branch small chain (needs all of x) ----
            gsum = sm.tile([C, 1], f32)
            nc.vector.tensor_reduce(gsum, gsums, X, Alu.add)
            ss_ps = sps.tile([1, 1], f32, tag="sp")
            nc.tensor.matmul(ss_ps, gsum, gsum, start=True, stop=True)
            ss = sm.tile([1, 1], f32)
            nc.vector.tensor_copy(ss, ss_ps)
            rn2 = sm.tile([1, 1], f32)
            nc.vector.reciprocal(rn2, ss)          # 1 / ||gsum||^2

            v_ps = sps.tile([C, 1], f32, tag="sp")
            nc.tensor.matmul(v_ps, wcho, gsum, start=True, stop=True)
            v_sb = sm.tile([C, 1], f32)
            nc.vector.tensor_copy(v_sb, v_ps)
            pqr = sm.tile([C, 3], f32)
            for j in range(3):
                nc.vector.tensor_tensor(pqr[:, j:j + 1], v_sb, taps[:, j:j + 1],
                                        Alu.mult)
            pqr2_ps = sps.tile([C, 3], f32, tag="sp")
            nc.tensor.matmul(pqr2_ps, wchi, pqr, start=True, stop=True)
            pqr2 = sm.tile([C, 3], f32)
            nc.vector.tensor_copy(pqr2, pqr2_ps)

            u_ps = sps.tile([1, 32], f32, tag="sp")
            nc.tensor.matmul(u_ps, gsum, wm1t, start=True, stop=True)
            u_sb = sm.tile([1, 32], f32)
            nc.vector.tensor_copy(u_sb, u_ps)
            u_sc = sm.tile([1, 32], f32)
            nc.vector.tensor_scalar(u_sc, u_sb, rn2[0:1, 0:1], None, Alu.mult)
            # broadcast [1,32] -> [C,32] with a K=1 ones matmul (no DRAM trip)
            ub_ps = sps.tile([C, 32], f32, tag="sp")
            nc.tensor.matmul(ub_ps, ones_row, u_sc, start=True, stop=True)
            u_bc = sm.tile([C, 32], f32)
            nc.vector.tensor_copy(u_bc, ub_ps)

            mqt = sm.tile([C, 32], bf16)
            mpt = sm.tile([C, 32], bf16)
            mrt = sm.tile([C, 32], bf16)
            for t, j in [(mpt, 0), (mqt, 1), (mrt, 2)]:
                nc.vector.tensor_scalar(t, u_bc, pqr2[:, j:j + 1], None,
                                        Alu.mult)

            # ---- streaming main loop ----
            def h1_group(k):
                h1ps = hps.tile([C, NCH], f32)
                for wgt, shift in [(mqt, 0), (mpt, -1), (mrt, +1)]:
                    for j in range(4):
                        n = 4 * k + j
                        n0 = n * NCH
                        lo = n0 + shift
                        hi = n0 + NCH + shift
                        plo, phi = 0, NCH
                        if lo < 0:
                            plo, lo = 1, 0
                        if hi > L:
                            phi, hi = NCH - 1, L
                        nc.tensor.matmul(
                            h1ps[32 * j:32 * j + 32, plo:phi],
                            wgt[:, 0:32], xbf[:, lo:hi],
                            start=(shift == 0), stop=(shift == 1),
                            tile_position=(0, 32 * j))
                nc.scalar.activation(h1sb[:, k * NCH:(k + 1) * NCH], h1ps,
                                     Act.Silu, bias=b1t[:, 0:1])

            def dr_pair(base, istride, n):
                """fp8 ifmap AP reading, for each of n cols j, the K-half pair
                (x8[base+j], x8[base+istride+j]) for a DoubleRow matmul."""
                anchor = x8[:, base:base + 1]
                o = anchor.opt()
                return bass.AP(tensor=anchor.tensor, offset=o.offset,
                               ap=[list(o.ap[0]), [istride, 2], [1, n]])

            a2w = a2d.rearrange("p (i m) -> p i m", i=2)
            a2s = a2d[:, 0:C]   # single (non-DR) fp8 A2 view

            def window_arrays(g):
                """Window arrays: H, V of the diff branch (V-axis abs on
                the otherwise-idle GpSimd engine)."""
                G0 = g * NW
                dh = win.tile([C, NW + 1], bf16, tag="dh")
                H = win.tile([C, NW], bf16, tag="H")
                dv = win.tile([C, NW + 128], bf16, tag="dv")
                V = win.tile([C, NW], bf16, tag="V")

                # dh[j] = x[G0+j] - x[G0+j-1], j in [a, e); |.| in place
                a = 1 if g == 0 else 0
                e = NW if g == NGRP - 1 else NW + 1
                nc.vector.tensor_tensor(dh[:, a:e],
                                        xbf[:, G0 + a:G0 + e],
                                        xbf[:, G0 + a - 1:G0 + e - 1],
                                        Alu.subtract)
                dh2 = dh[:, 0:NW].rearrange("p (r c) -> p r c", c=Wimg)
                nc.vector.memset(dh2[:, :, 0:1], 0.0)   # no cross-row diffs
                nc.vector.memset(dh[:, NW:NW + 1], 0.0)
                dhu = dh.bitcast(u16)
                nc.vector.tensor_scalar(dhu[:, a:e], dhu[:, a:e], 0x7FFF,
                                        None, Alu.bitwise_and)
                # H[j] = |dh[j]| + |dh[j+1]|
                nc.vector.tensor_tensor(H, dh[:, 0:NW], dh[:, 1:NW + 1],
                                        Alu.add)
                # edge fix: col0 += |dh[row,1]| ; col127 += |dh[row,127]|
                H2 = H.rearrange("p (r c) -> p r c", c=Wimg)
                nc.vector.tensor_tensor(H2[:, :, 0:1], H2[:, :, 0:1],
                                        dh2[:, :, 1:2], Alu.add)
                nc.vector.tensor_tensor(H2[:, :, Wimg - 1:Wimg],
                                        H2[:, :, Wimg - 1:Wimg],
                                        dh2[:, :, Wimg - 1:Wimg], Alu.add)

                # dv[j] = x[G0+j] - x[G0+j-128]; abs on GpSimd
                av = 128 if g == 0 else 0
                ev = NW if g == NGRP - 1 else NW + 128
                nc.vector.tensor_tensor(dv[:, av:ev], xbf[:, G0 + av:G0 + ev],
                                        xbf[:, G0 + av - 128:G0 + ev - 128],
                                        Alu.subtract)
                dvu = dv.bitcast(u16)
                nc.vector.tensor_scalar(dvu[:, av:ev], dvu[:, av:ev], 0x7FFF,
                                        None, Alu.bitwise_and)
                if g == 0:
                    nc.vector.memset(dv[:, 0:128], 0.0)   # row 0: no up-diff
                if g == NGRP - 1:
                    # last row reflect: pair partner := own value -> 2|dv|
                    nc.vector.tensor_copy(dv[:, NW:NW + 128],
                                          dv[:, NW - 128:NW])
                # V[j] = |dv[j]| + |dv[j+128]|
                nc.vector.tensor_tensor(V, dv[:, 0:NW], dv[:, 128:NW + 128],
                                        Alu.add)
                if g == 0:
                    # row 0 reflect: V = 2*|dv[j+128]|
                    nc.vector.tensor_tensor(V[:, 0:128], V[:, 0:128],
                                            dv[:, 128:256], Alu.add)
                return H, V

            def chunk_pair(m, H, V):
                """Chunks 2m, 2m+1 into one 2-bank psum tile; the bf16
                matmuls (B3, Wd@H, Wd@V) run 1024 wide."""
                ps = yps.tile([C, 2 * NCH], f32)
                NP = 2 * NCH
                p0 = 2 * m * NCH          # first flat column of the pair
                off = (2 * m % 4) * NCH   # H/V window-local offset
                for h in range(2):
                    nc.tensor.matmul(ps[:, h * NCH:(h + 1) * NCH], wb3t,
                                     xbf[:, p0 + h * NCH:p0 + (h + 1) * NCH],
                                     start=True, stop=False)
                for h in range(2):
                    n = 2 * m + h
                    n0 = n * NCH
                    q = h * NCH
                    # A2 @ (x[l-1]+x[l+1]) as one fp8 DoubleRow matmul
                    plo = 1 if n == 0 else 0
                    phi = NCH - 1 if n == NCHUNK - 1 else NCH
                    nc.tensor.matmul(ps[:, q + plo:q + phi], a2w,
                                     dr_pair(n0 + plo - 1, 2, phi - plo),
                                     start=False, stop=False, perf_mode=DR)
                    if n == 0:      # l=0 keeps only the right neighbor
                        nc.tensor.matmul(ps[:, q:q + 1], a2s, x8[:, 1:2],
                                         start=False, stop=False)
                    if n == NCHUNK - 1:   # l=L-1 keeps only the left
                        nc.tensor.matmul(ps[:, q + NCH - 1:q + NCH], a2s,
                                         x8[:, L - 2:L - 1],
                                         start=False, stop=False)
                    # A2 @ (x[l-128]+x[l+128]) as one fp8 DoubleRow matmul
                    vlo = 128 if n == 0 else 0
                    vhi = NCH - 128 if n == NCHUNK - 1 else NCH
                    nc.tensor.matmul(ps[:, q + vlo:q + vhi], a2w,
                                     dr_pair(n0 + vlo - 128, 256, vhi - vlo),
                                     start=False, stop=False, perf_mode=DR)
                    if n == 0:      # first image row keeps only down
                        nc.tensor.matmul(ps[:, q:q + 128], a2s,
                                         x8[:, 128:256],
                                         start=False, stop=False)
                    if n == NCHUNK - 1:   # last image row keeps only up
                        nc.tensor.matmul(ps[:, q + NCH - 128:q + NCH], a2s,
                                         x8[:, L - 256:L - 128],
                                         start=False, stop=False)
                    if n == 0:
                        # col-scan wrap: l=j gets x[(h-1)w + j - 1]
                        nc.tensor.matmul(ps[:, q + 1:q + 128], wa2t,
                                         xbf[:, L - Wimg:L - 1],
                                         start=False, stop=False)
                    if n == NCHUNK - 1:
                        # col-scan wrap: l=(h-1)w+j gets x[j+1]
                        nc.tensor.matmul(ps[:, q + NCH - 128:q + NCH - 1],
                                         wa2t, xbf[:, 1:128],
                                         start=False, stop=False)
                # diff branch
                for h in range(2):
                    pso = ps[:, h * NCH:(h + 1) * NCH]
                    o2 = off + h * NCH
                    nc.tensor.matmul(pso, wdt, H[:, o2:o2 + NCH],
                                     start=False, stop=False)
                    nc.tensor.matmul(pso, wdt, V[:, o2:o2 + NCH],
                                     start=False, stop=False)
                # channel contribution (row-tiled, K=32)
                for h in range(2):
                    n = 2 * m + h
                    j = n % 4
                    nc.tensor.matmul(ps[:, h * NCH:h * NCH + NCH],
                                     c2t4[32 * j:32 * j + 32, :],
                                     h1sb[32 * j:32 * j + 32,
                                          (n // 4) * NCH:(n // 4 + 1) * NCH],
                                     start=False, stop=(h == 1),
                                     tile_position=(32 * j, 0))
                nc.scalar.activation(ypre[:, p0:p0 + NP], ps,
                                     Act.Identity, bias=bout[:, 0:1],
                                     accum_out=ysum[:, m:m + 1])
                dmp = dump.tile([C, NP], bf16, tag="sq")
                nc.scalar.activation(dmp, ps, Act.Square,
                                     accum_out=ysq[:, m:m + 1])

            h1_group(0)
            win_arrays = window_arrays(0)
            for k in range(NGRP):
                nxt = None
                if k + 1 < NGRP:
                    h1_group(k + 1)
                    nxt = window_arrays(k + 1)
                chunk_pair(2 * k, *win_arrays)
                chunk_pair(2 * k + 1, *win_arrays)
                win_arrays = nxt

            # ---- global BN stats via AllReduce ----
            # ysq tracked z = y - bout (PSUM, pre-bias):
            #   sum(y^2) = sum(z^2) + 2*bout*sum(y) - L*bout^2
            stats = sm.tile([C, 2], f32)
            nc.vector.tensor_reduce(stats[:, 0:1], ysum, X, Alu.add)
            nc.vector.tensor_reduce(stats[:, 1:2], ysq, X, Alu.add)
            cb = sm.tile([C, 1], f32)
            nc.vector.tensor_tensor(cb, bout, stats[:, 0:1], Alu.mult)
            nc.vector.scalar_tensor_tensor(stats[:, 1:2], cb, 2.0,
                                           stats[:, 1:2], Alu.mult, Alu.add)
            bsq = sm.tile([C, 1], f32)
            nc.vector.tensor_tensor(bsq, bout, bout, Alu.mult)
            nc.vector.scalar_tensor_tensor(stats[:, 1:2], bsq, -float(L),
                                           stats[:, 1:2], Alu.mult, Alu.add)
            # prefetch the sqrt ACT table while the collective runs
            sqpre = sm.tile([C, 1], f32)
            nc.scalar.activation(sqpre, stats[:, 1:2], Act.Sqrt)
            cc_in = dram.tile([C, 2], f32)
            cc_out = dram.tile([C, 2], f32)
            nc.gpsimd.dma_start(out=cc_in[:], in_=stats)
            nc.gpsimd.collective_compute(
                "AllReduce", Alu.add,
                replica_groups=[list(range(NCORES))],
                ins=[cc_in.opt()], outs=[cc_out.opt()])
            statsr = sm.tile([C, 2], f32)
            nc.gpsimd.dma_start(out=statsr, in_=cc_out[:])

            mean = sm.tile([C, 1], f32)
            ex2 = sm.tile([C, 1], f32)
            nc.vector.tensor_scalar(mean, statsr[:, 0:1], 1.0 / NTOT, None,
                                    Alu.mult)
            nc.vector.tensor_scalar(ex2, statsr[:, 1:2], 1.0 / NTOT, None,
                                    Alu.mult)
            m2 = sm.tile([C, 1], f32)
            nc.vector.tensor_tensor(m2, mean, mean, Alu.mult)
            varep = sm.tile([C, 1], f32)
            nc.vector.tensor_tensor(varep, ex2, m2, Alu.subtract)
            nc.vector.tensor_scalar(varep, varep, EPS_BN, None, Alu.add)
            inv = sm.tile([C, 1], f32)
            nc.vector.reciprocal(inv, varep)
            rstd = sm.tile([C, 1], f32)
            nc.scalar.activation(rstd, inv, Act.Sqrt)
            s_sc = sm.tile([C, 1], f32)
            nc.vector.tensor_tensor(s_sc, rstd, gb[:, 0:1], Alu.mult)
            ms = sm.tile([C, 1], f32)
            nc.vector.tensor_tensor(ms, mean, s_sc, Alu.mult)
            t_sc = sm.tile([C, 1], f32)
            nc.vector.tensor_tensor(t_sc, gb[:, 1:2], ms, Alu.subtract)

            # ---- apply BN (DVE 2x, bf16 out), write out on two queues ----
            for g in range(NGRP):
                lo, hi = g * NW, (g + 1) * NW
                ow = owp.tile([C, NW], bf16, tag="ow")
                nc.vector.tensor_scalar(ow, ypre[:, lo:hi],
                                        s_sc[:, 0:1], t_sc[:, 0:1],
                                        Alu.mult, Alu.add)
                eng = nc.sync if g % 2 == 0 else nc.scalar
                eng.dma_start(out=y_ext[:, lo:hi], in_=ow)

    _split_excess_waits(nc)
    return nc


def _fold_weights(inputs):
    f = np.float32
    W_in = inputs["w_spatial_in"].astype(np.float64)
    W_out = inputs["w_spatial_out"].astype(np.float64)
    dw_sp = inputs["w_dw_spatial"][:, 0, :].astype(np.float64)
    W_proj = inputs["w_out_proj"].astype(np.float64)
    W_mlp2 = inputs["w_mlp2"].astype(np.float64)
    dwt = float(inputs["diff_weight"])

    a_sym = dw_sp[:, 0] + dw_sp[:, 2]
    w1 = dw_sp[:, 1]
    A2 = 0.25 * W_proj @ (W_out * a_sym[None, :]) @ W_in
    B3 = W_proj @ (W_out * w1[None, :]) @ W_in + W_proj
    W_d = 0.25 * dwt * W_proj
    C2 = W_proj @ W_mlp2                     # [c, 32]
    bias_out = W_proj @ inputs["b_mlp2"].astype(np.float64)

    bf = ml_dtypes.bfloat16
    f8 = ml_dtypes.float8_e4m3
    a2t8 = np.ascontiguousarray((A2.T * ASC).astype(f8))
    return {
        "wb3t": np.ascontiguousarray(B3.T.astype(bf)),
        "a2d": np.ascontiguousarray(np.concatenate([a2t8, a2t8], axis=1)),
        "wa2t": np.ascontiguousarray(A2.T.astype(bf)),
        "wdt": np.ascontiguousarray(W_d.T.astype(bf)),
        "c2t4": np.ascontiguousarray(np.tile(C2.T.astype(bf), (4, 1))),
        "wcho": np.ascontiguousarray(inputs["w_ch_out"].astype(f)),
        "wchi": np.ascontiguousarray(inputs["w_ch_in"].astype(f)),
        "wm1t": np.ascontiguousarray(inputs["w_mlp1"].T.astype(f)),
        "taps": np.ascontiguousarray(inputs["w_ch_dw"][:, 0, :].astype(f)),
        "b1t": np.ascontiguousarray(
            np.tile(inputs["b_mlp1"].astype(f), 4)[:, None]),
        "bout": np.ascontiguousarray(bias_out.astype(f)[:, None]),
        "gb": np.ascontiguousarray(
            np.stack([inputs["bn_gamma"], inputs["bn_beta"]], 1).astype(f)),
    }


def prepare_in_maps(inputs):
    wmap = _fold_weights(inputs)
    x = np.asarray(inputs["x"]).astype(np.float32)  # [B, C, H, W]
    in_maps = []
    for b in range(NCORES):
        m = dict(wmap)
        xb = x[b].reshape(C, L)
        m["x"] = np.ascontiguousarray(xb.astype(ml_dtypes.bfloat16))
        m["x8"] = np.ascontiguousarray(
            (xb * XSC).astype(ml_dtypes.float8_e4m3))
        in_maps.append(m)
    return in_maps


def kernel(**inputs):
    from concourse.bass_utils import run_bass_kernel_spmd

    inputs = {k: np.asarray(v) for k, v in inputs.items()}
    if "nc" not in _CACHE:
        _CACHE["nc"] = _build_program()
    nc = _CACHE["nc"]

    in_maps = prepare_in_maps(inputs)
    res = run_bass_kernel_spmd(nc, in_maps, list(range(NCORES)))
    out = np.stack([np.asarray(res.results[b]["y"]).astype(np.float32)
                    .reshape(C, Himg, Wimg) for b in range(NCORES)])
    return out


# revision 16
# speedup vs baseline: 1.3242x; 1.0659x over previous
"""OCS fused kernel for Trainium2, data-parallel over batch across 8 cores.

Algebraic restructuring (verified vs reference to ~1e-6 in fp64):

Spatial branch (4 scan orders, shared weights) collapses to a symmetric
5-point stencil with scan-order wrap rules, and the two 1x1 convs fold
through it:  W_proj @ y_sp = A2 @ (4-neighbor sum of x) + (B3 - W_proj) @ x.
The 4-neighbor sums are not materialized: A2 is applied as two fp8
DoubleRow matmuls, each fusing a +-shift pair of x (second K-half read via
a strided AP view), with the A2 magnitude rescaled 2^4 into fp8 range and
x pre-scaled 2^-4 on host (A2 term is ~0.7% of y, fp8 error is negligible
there). Col-scan wraps are two extra small bf16 matmuls.
Channel branch: m = g g^T is rank-1, so the whole conv pipeline collapses
into three [32,128] matmuls on shifted x (weights MP/MQ/MR = u (x) P/Q/R
built on-device from g = sum of x), a silu, and one [128,32] matmul.
Diff branch (large contributor -> bf16): |dx| per axis once (H-axis abs on
DVE, V-axis abs offloaded to GpSimd), pair-sums as shifted adds, W_proj
folds in as W_d @ H + W_d @ V.
BatchNorm: per-core partial (sum, sumsq) -> 1KB AllReduce (warmed up by a
dummy collective at kernel start so the mesh-algo load is off the critical
path) -> affine applied at DVE 2x with bf16 output (host upcasts to f32).
"""

import numpy as np
import ml_dtypes

B, C, Himg, Wimg = 8, 128, 128, 128
L = Himg * Wimg            # 16384
NCORES = 8
NCH = 512                  # matmul chunk columns
NCHUNK = L // NCH          # 32
NPAIR = NCHUNK // 2        # chunk pairs -> 2-bank psum tiles
NW = 2048                  # elementwise window columns (4 chunks)
NGRP = L // NW             # 8
NROW = NW // Wimg          # image rows per window (16)
EPS_BN = 1e-5
NTOT = float(B * L)        # batchnorm population per channel
XSC = 2.0 ** -4            # host prescale of the fp8 x copy
ASC = 2.0 ** 4             # fp8 A2 weight upscale (cancels XSC)

_CACHE = {}


def _make_patched_tc():
    """TileContext whose exit drain splits sem waits one-per-Drain.

    The walrus build in this container rejects Drain instructions carrying
    more than one sem wait ("Too many sync wait commands"). Stock
    TileContext attaches the whole global vector clock to a single tail
    Drain; emit one Drain per outstanding proc instead.
    """
    import bass_rust
    import concourse.tile as tile
    from concourse.vector_clock import ScopedClock

    class PatchedTC(tile.TileContext):
        def _drain_and_barrier(self, tick_clock, wait_clock):
            gc = list(tick_clock.global_clock)
            for i, v in enumerate(gc):
                if v:
                    single = [0] * len(gc)
                    single[i] = v
                    d = self.nc.sync.drain()
                    wait_clock.add_sem_waits(
                        d.ins, ScopedClock({None: bass_rust.VectorClock(single)})
                    )
            self.nc.all_engine_barrier()
            assert self.sems is not None
            popped = self.nc._tile_sem_poison_stack.pop()
            assert popped is self._sem_poison
            self.nc.clear_and_free_semaphores(list(self.sems.allocated().values()))
            self.nc.all_engine_barrier()

    return PatchedTC


def _split_excess_waits(nc):
    """Walrus here allows one sem wait per instruction; hoist extras onto
    same-engine NoOps inserted immediately before the instruction."""
    import bass_rust

    nid = 0
    for blk in nc.main_func.blocks:
        out = []
        for ins in blk.instructions:
            si = getattr(ins, "sync_info", None)
            waits = list(si.on_wait) if si is not None else []
            if len(waits) > 1:
                for w in waits[:-1]:
                    nid += 1
                    nop = bass_rust.InstNoOp(
                        name=f"I-waitsplit-{nid}", ins=[], outs=[])
                    nop.engine = ins.engine
                    nop.sync_info = bass_rust.SyncInfo(
                        on_wait=[w], on_update=[])
                    nc.register_instruction(nop, overwrite=True)
                    out.append(nop)
                si.on_wait = [waits[-1]]
                ins.sync_info = si
            out.append(ins)
        blk.instructions = out


def _build_program():
    import concourse.bass as bass
    import concourse.mybir as mybir

    PatchedTC = _make_patched_tc()

    f32 = mybir.dt.float32
    bf16 = mybir.dt.bfloat16
    fp8 = mybir.dt.float8e4
    u16 = mybir.dt.uint16
    Alu = mybir.AluOpType
    Act = mybir.ActivationFunctionType
    X = mybir.AxisListType.X
    DR = mybir.MatmulPerfMode.DoubleRow

    nc = bass.Bass(target_bir_lowering=False, num_devices=NCORES)

    x_ext = nc.declare_dram_parameter("x", [C, L], bf16, isOutput=False)
    x8_ext = nc.declare_dram_parameter("x8", [C, L], fp8, isOutput=False)
    # packed weights: one DMA per dtype class
    wbf_ext = nc.declare_dram_parameter("wbf", [C, 4 * C], bf16,
                                        isOutput=False)
    a2d_ext = nc.declare_dram_parameter("a2d", [C, 2 * C], fp8, isOutput=False)
    wf32_ext = nc.declare_dram_parameter("wf32", [C, 2 * C + 39], f32,
                                         isOutput=False)
    y_ext = nc.declare_dram_parameter("y", [C, L], bf16, isOutput=True)

    with PatchedTC(nc) as tc:
        with (
            tc.tile_pool(name="wp", bufs=1) as wp,
            tc.tile_pool(name="big", bufs=1) as big,
            tc.tile_pool(name="win", bufs=3) as win,
            tc.tile_pool(name="sm", bufs=1) as sm,
            tc.tile_pool(name="dump", bufs=2) as dump,
            tc.tile_pool(name="ow", bufs=4) as owp,
            tc.tile_pool(name="yps", bufs=3, space="PSUM") as yps,
            tc.tile_pool(name="hps", bufs=1, space="PSUM") as hps,
            tc.tile_pool(name="sps", bufs=1, space="PSUM") as sps,
            tc.tile_pool(name="dram", bufs=1, space="DRAM") as dram,
        ):
            # ---- weights to SBUF (gpsimd queue; x on scalar/sync queues) ----
            wbf = wp.tile([C, 4 * C], bf16)
            a2d = wp.tile([C, 2 * C], fp8)
            wf32 = wp.tile([C, 2 * C + 39], f32)
            wb3t = wbf[:, 0:C]
            wa2t = wbf[:, C:2 * C]
            wdt = wbf[:, 2 * C:3 * C]
            c2t4 = wbf[:, 3 * C:4 * C]
            wcho = wf32[:, 0:C]
            wchi = wf32[:, C:2 * C]
            wm1t = wf32[:, 2 * C:2 * C + 32]
            taps = wf32[:, 2 * C + 32:2 * C + 35]
            b1t = wf32[:, 2 * C + 35:2 * C + 36]
            bout = wf32[:, 2 * C + 36:2 * C + 37]
            gb = wf32[:, 2 * C + 37:2 * C + 39]
            ones_row = wp.tile([1, C], f32)
            nc.vector.memset(ones_row, 1.0)
            for t, e in [(wbf, wbf_ext), (a2d, a2d_ext), (wf32, wf32_ext)]:
                nc.gpsimd.dma_start(out=t, in_=e[:])

            # warmup collective: loads the CC mesh algo while compute runs,
            # so the real stats AllReduce skips the ~11us startup
            ccw_in = dram.tile([C, 2], f32)
            ccw_out = dram.tile([C, 2], f32)
            nc.gpsimd.dma_start(out=ccw_in[:], in_=gb)
            nc.gpsimd.collective_compute(
                "AllReduce", Alu.add,
                replica_groups=[list(range(NCORES))],
                ins=[ccw_in.opt()], outs=[ccw_out.opt()])

            # ---- big SBUF arrays ----
            xbf = big.tile([C, L], bf16)     # x (bf16, cast on host)
            x8 = big.tile([C, L], fp8)       # x * 2^-4 (fp8, cast on host)
            ypre = big.tile([C, L], bf16)    # pre-BN output
            h1sb = big.tile([C, NGRP * NCH], bf16)  # silu(h1) packed 4ch/grp

            gsums = sm.tile([C, NGRP], f32)
            ysum = sm.tile([C, NPAIR], f32)
            ysq = sm.tile([C, NPAIR], f32)

            # ---- load x split across two DMA queues (window 7 early:
            #      chunk 0's wrap matmul reads it) ----
            for g in [0, 7, 1, 6]:
                lo, hi = g * NW, (g + 1) * NW
                nc.scalar.dma_start(out=xbf[:, lo:hi], in_=x_ext[:, lo:hi])
            for g in [2, 3, 4, 5]:
                lo, hi = g * NW, (g + 1) * NW
                nc.sync.dma_start(out=xbf[:, lo:hi], in_=x_ext[:, lo:hi])
            nc.sync.dma_start(out=x8, in_=x8_ext[:])
            # preload the ACT function tables while the x transfers run
            tdum = sm.tile([1, 1], f32)
            for fn in (Act.Copy, Act.Identity, Act.Square, Act.Silu,
                       Act.Sqrt):
                nc.scalar.activation(tdum, ones_row[0:1, 0:1], fn)
            # per-window row sums: 4 windows on ACT, 4 on DVE (both idle now)
            for g in [0, 7, 1, 6]:
                lo, hi = g * NW, (g + 1) * NW
                gdump = dump.tile([C, NW], bf16, tag="gs")
                nc.scalar.activation(gdump, xbf[:, lo:hi], Act.Copy,
                                     accum_out=gsums[:, g:g + 1])
            for g in [2, 3, 4, 5]:
                lo, hi = g * NW, (g + 1) * NW
                nc.vector.tensor_reduce(gsums[:, g:g + 1], xbf[:, lo:hi],
                                        X, Alu.add)

            # ---- channel-branch small chain (needs all of x) ----
            gsum = sm.tile([C, 1], f32)
            nc.vector.tensor_reduce(gsum, gsums, X, Alu.add)
            ss_ps = sps.tile([1, 1], f32, tag="sp")
            nc.tensor.matmul(ss_ps, gsum, gsum, start=True, stop=True)
            ss = sm.tile([1, 1], f32)
            nc.vector.tensor_copy(ss, ss_ps)
            rn2 = sm.tile([1, 1], f32)
            nc.vector.reciprocal(rn2, ss)          # 1 / ||gsum||^2

            v_ps = sps.tile([C, 1], f32, tag="sp")
            nc.tensor.matmul(v_ps, wcho, gsum, start=True, stop=True)
            v_sb = sm.tile([C, 1], f32)
            nc.vector.tensor_copy(v_sb, v_ps)
            pqr = sm.tile([C, 3], f32)
            for j in range(3):
                nc.vector.tensor_tensor(pqr[:, j:j + 1], v_sb, taps[:, j:j + 1],
                                        Alu.mult)
            pqr2_ps = sps.tile([C, 3], f32, tag="sp")
            nc.tensor.matmul(pqr2_ps, wchi, pqr, start=True, stop=True)
            pqr2 = sm.tile([C, 3], f32)
            nc.vector.tensor_copy(pqr2, pqr2_ps)

            u_ps = sps.tile([1, 32], f32, tag="sp")
            nc.tensor.matmul(u_ps, gsum, wm1t, start=True, stop=True)
            u_sb = sm.tile([1, 32], f32)
            nc.vector.tensor_copy(u_sb, u_ps)
            u_sc = sm.tile([1, 32], f32)
            nc.vector.tensor_scalar(u_sc, u_sb, rn2[0:1, 0:1], None, Alu.mult)
            # broadcast [1,32] -> [C,32] with a K=1 ones matmul (no DRAM trip)
            ub_ps = sps.tile([C, 32], f32, tag="sp")
            nc.tensor.matmul(ub_ps, ones_row, u_sc, start=True, stop=True)
            u_bc = sm.tile([C, 32], f32)
            nc.vector.tensor_copy(u_bc, ub_ps)

            mqt = sm.tile([C, 32], bf16)
            mpt = sm.tile([C, 32], bf16)
            mrt = sm.tile([C, 32], bf16)
            for t, j in [(mpt, 0), (mqt, 1), (mrt, 2)]:
                nc.vector.tensor_scalar(t, u_bc, pqr2[:, j:j + 1], None,
                                        Alu.mult)

            # ---- streaming main loop ----
            def h1_group(k):
                h1ps = hps.tile([C, NCH], f32)
                for wgt, shift in [(mqt, 0), (mpt, -1), (mrt, +1)]:
                    for j in range(4):
                        n = 4 * k + j
                        n0 = n * NCH
                        lo = n0 + shift
                        hi = n0 + NCH + shift
                        plo, phi = 0, NCH
                        if lo < 0:
                            plo, lo = 1, 0
                        if hi > L:
                            phi, hi = NCH - 1, L
                        nc.tensor.matmul(
                            h1ps[32 * j:32 * j + 32, plo:phi],
                            wgt[:, 0:32], xbf[:, lo:hi],
                            start=(shift == 0), stop=(shift == 1),
                            tile_position=(0, 32 * j))
                nc.scalar.activation(h1sb[:, k * NCH:(k + 1) * NCH], h1ps,
                                     Act.Silu, bias=b1t[:, 0:1])

            def dr_pair(base, istride, n):
                """fp8 ifmap AP reading, for each of n cols j, the K-half pair
                (x8[base+j], x8[base+istride+j]) for a DoubleRow matmul."""
                anchor = x8[:, base:base + 1]
                o = anchor.opt()
                return bass.AP(tensor=anchor.tensor, offset=o.offset,
                               ap=[list(o.ap[0]), [istride, 2], [1, n]])

            a2w = a2d.rearrange("p (i m) -> p i m", i=2)
            a2s = a2d[:, 0:C]   # single (non-DR) fp8 A2 view

            def window_arrays(g):
                """Window arrays: H, V of the diff branch (V-axis abs on
                the otherwise-idle GpSimd engine)."""
                G0 = g * NW
                dh = win.tile([C, NW + 1], bf16, tag="dh")
                H = win.tile([C, NW], bf16, tag="H")
                dv = win.tile([C, NW + 128], bf16, tag="dv")
                V = win.tile([C, NW], bf16, tag="V")

                # dh[j] = x[G0+j] - x[G0+j-1], j in [a, e); |.| in place
                a = 1 if g == 0 else 0
                e = NW if g == NGRP - 1 else NW + 1
                nc.vector.tensor_tensor(dh[:, a:e],
                                        xbf[:, G0 + a:G0 + e],
                                        xbf[:, G0 + a - 1:G0 + e - 1],
                                        Alu.subtract)
                dh2 = dh[:, 0:NW].rearrange("p (r c) -> p r c", c=Wimg)
                nc.vector.memset(dh2[:, :, 0:1], 0.0)   # no cross-row diffs
                nc.vector.memset(dh[:, NW:NW + 1], 0.0)
                dhu = dh.bitcast(u16)
                nc.vector.tensor_scalar(dhu[:, a:e], dhu[:, a:e], 0x7FFF,
                                        None, Alu.bitwise_and)
                # H[j] = |dh[j]| + |dh[j+1]|
                nc.vector.tensor_tensor(H, dh[:, 0:NW], dh[:, 1:NW + 1],
                                        Alu.add)
                # edge fix: col0 += |dh[row,1]| ; col127 += |dh[row,127]|
                H2 = H.rearrange("p (r c) -> p r c", c=Wimg)
                nc.vector.tensor_tensor(H2[:, :, 0:1], H2[:, :, 0:1],
                                        dh2[:, :, 1:2], Alu.add)
                nc.vector.tensor_tensor(H2[:, :, Wimg - 1:Wimg],
                                        H2[:, :, Wimg - 1:Wimg],
                                        dh2[:, :, Wimg - 1:Wimg], Alu.add)

                # dv[j] = x[G0+j] - x[G0+j-128]; abs on GpSimd
                av = 128 if g == 0 else 0
                ev = NW if g == NGRP - 1 else NW + 128
                nc.vector.tensor_tensor(dv[:, av:ev], xbf[:, G0 + av:G0 + ev],
                                        xbf[:, G0 + av - 128:G0 + ev - 128],
                                        Alu.subtract)
                dvu = dv.bitcast(u16)
                nc.vector.tensor_scalar(dvu[:, av:ev], dvu[:, av:ev], 0x7FFF,
                                        None, Alu.bitwise_and)
                if g == 0:
                    nc.vector.memset(dv[:, 0:128], 0.0)   # row 0: no up-diff
                if g == NGRP - 1:
                    # last row reflect: pair partner := own value -> 2|dv|
                    nc.vector.tensor_copy(dv[:, NW:NW + 128],
                                          dv[:, NW - 128:NW])
                # V[j] = |dv[j]| + |dv[j+128]|
                nc.vector.tensor_tensor(V, dv[:, 0:NW], dv[:, 128:NW + 128],
                                        Alu.add)
                if g == 0:
                    # row 0 reflect: V = 2*|dv[j+128]|
                    nc.vector.tensor_tensor(V[:, 0:128], V[:, 0:128],
                                            dv[:, 128:256], Alu.add)
                return H, V

            def chunk_pair(m, H, V):
                """Chunks 2m, 2m+1 into one 2-bank psum tile; the bf16
                matmuls (B3, Wd@H, Wd@V) run 1024 wide."""
                ps = yps.tile([C, 2 * NCH], f32)
                NP = 2 * NCH
                p0 = 2 * m * NCH          # first flat column of the pair
                off = (2 * m % 4) * NCH   # H/V window-local offset
                for h in range(2):
                    nc.tensor.matmul(ps[:, h * NCH:(h + 1) * NCH], wb3t,
                                     xbf[:, p0 + h * NCH:p0 + (h + 1) * NCH],
                                     start=True, stop=False)
                for h in range(2):
                    n = 2 * m + h
                    n0 = n * NCH
                    q = h * NCH
                    # A2 @ (x[l-1]+x[l+1]) as one fp8 DoubleRow matmul
                    plo = 1 if n == 0 else 0
                    phi = NCH - 1 if n == NCHUNK - 1 else NCH
                    nc.tensor.matmul(ps[:, q + plo:q + phi], a2w,
                                     dr_pair(n0 + plo - 1, 2, phi - plo),
                                     start=False, stop=False, perf_mode=DR)
                    if n == 0:      # l=0 keeps only the right neighbor
                        nc.tensor.matmul(ps[:, q:q + 1], a2s, x8[:, 1:2],
                                         start=False, stop=False)
                    if n == NCHUNK - 1:   # l=L-1 keeps only the left
                        nc.tensor.matmul(ps[:, q + NCH - 1:q + NCH], a2s,
                                         x8[:, L - 2:L - 1],
                                         start=False, stop=False)
                    # A2 @ (x[l-128]+x[l+128]) as one fp8 DoubleRow matmul
                    vlo = 128 if n == 0 else 0
                    vhi = NCH - 128 if n == NCHUNK - 1 else NCH
                    nc.tensor.matmul(ps[:, q + vlo:q + vhi], a2w,
                                     dr_pair(n0 + vlo - 128, 256, vhi - vlo),
                                     start=False, stop=False, perf_mode=DR)
                    if n == 0:      # first image row keeps only down
                        nc.tensor.matmul(ps[:, q:q + 128], a2s,
                                         x8[:, 128:256],
                                         start=False, stop=False)
                    if n == NCHUNK - 1:   # last image row keeps only up
                        nc.tensor.matmul(ps[:, q + NCH - 128:q + NCH], a2s,
                                         x8[:, L - 256:L - 128],
                                         start=False, stop=False)
                    if n == 0:
                        # col-scan wrap: l=j gets x[(h-1)w + j - 1]
                        nc.tensor.matmul(ps[:, q + 1:q + 128], wa2t,
                                         xbf[:, L - Wimg:L - 1],
                                         start=False, stop=False)
                    if n == NCHUNK - 1:
                        # col-scan wrap: l=(h-1)w+j gets x[j+1]
                        nc.tensor.matmul(ps[:, q + NCH - 128:q + NCH - 1],
                                         wa2t, xbf[:, 1:128],
                                         start=False, stop=False)
                # diff branch
                for h in range(2):
                    pso = ps[:, h * NCH:(h + 1) * NCH]
                    o2 = off + h * NCH
                    nc.tensor.matmul(pso, wdt, H[:, o2:o2 + NCH],
                                     start=False, stop=False)
                    nc.tensor.matmul(pso, wdt, V[:, o2:o2 + NCH],
                                     start=False, stop=False)
                # channel contribution (row-tiled, K=32)
                for h in range(2):
                    n = 2 * m + h
                    j = n % 4
                    nc.tensor.matmul(ps[:, h * NCH:h * NCH + NCH],
                                     c2t4[32 * j:32 * j + 32, :],
                                     h1sb[32 * j:32 * j + 32,
                                          (n // 4) * NCH:(n // 4 + 1) * NCH],
                                     start=False, stop=(h == 1),
                                     tile_position=(32 * j, 0))
                nc.scalar.activation(ypre[:, p0:p0 + NP], ps,
                                     Act.Identity, bias=bout[:, 0:1],
                                     accum_out=ysum[:, m:m + 1])
                dmp = dump.tile([C, NP], bf16, tag="sq")
                nc.scalar.activation(dmp, ps, Act.Square,
                                     accum_out=ysq[:, m:m + 1])

            h1_group(0)
            win_arrays = window_arrays(0)
            for k in range(NGRP):
                nxt = None
                if k + 1 < NGRP:
                    h1_group(k + 1)
                    nxt = window_arrays(k + 1)
                chunk_pair(2 * k, *win_arrays)
                chunk_pair(2 * k + 1, *win_arrays)
                win_arrays = nxt

            # ---- global BN stats via AllReduce ----
            # ysq tracked z = y - bout (PSUM, pre-bias):
            #   sum(y^2) = sum(z^2) + 2*bout*sum(y) - L*bout^2
            stats = sm.tile([C, 2], f32)
            nc.vector.tensor_reduce(stats[:, 0:1], ysum, X, Alu.add)
            nc.vector.tensor_reduce(stats[:, 1:2], ysq, X, Alu.add)
            cb = sm.tile([C, 1], f32)
            nc.vector.tensor_tensor(cb, bout, stats[:, 0:1], Alu.mult)
            nc.vector.scalar_tensor_tensor(stats[:, 1:2], cb, 2.0,
                                           stats[:, 1:2], Alu.mult, Alu.add)
            bsq = sm.tile([C, 1], f32)
            nc.vector.tensor_tensor(bsq, bout, bout, Alu.mult)
            nc.vector.scalar_tensor_tensor(stats[:, 1:2], bsq, -float(L),
                                           stats[:, 1:2], Alu.mult, Alu.add)
            # prefetch the sqrt ACT table while the collective runs
            sqpre = sm.tile([C, 1], f32)
            nc.scalar.activation(sqpre, stats[:, 1:2], Act.Sqrt)
            cc_in = dram.tile([C, 2], f32)
            cc_out = dram.tile([C, 2], f32)
            nc.gpsimd.dma_start(out=cc_in[:], in_=stats)
            nc.gpsimd.collective_compute(
                "AllReduce", Alu.add,
                replica_groups=[list(range(NCORES))],
                ins=[cc_in.opt()], outs=[cc_out.opt()])
            statsr = sm.tile([C, 2], f32)
            nc.gpsimd.dma_start(out=statsr, in_=cc_out[:])

            mean = sm.tile([C, 1], f32)
            ex2 = sm.tile([C, 1], f32)
            nc.vector.tensor_scalar(mean, statsr[:, 0:1], 1.0 / NTOT, None,
                                    Alu.mult)
            nc.vector.tensor_scalar(ex2, statsr[:, 1:2], 1.0 / NTOT, None,
                                    Alu.mult)
            m2 = sm.tile([C, 1], f32)
            nc.vector.tensor_tensor(m2, mean, mean, Alu.mult)
            varep = sm.tile([C, 1], f32)
            nc.vector.tensor_tensor(varep, ex2, m2, Alu.subtract)
            nc.vector.tensor_scalar(varep, varep, EPS_BN, None, Alu.add)
            inv = sm.tile([C, 1], f32)
            nc.vector.reciprocal(inv, varep)
            rstd = sm.tile([C, 1], f32)
            nc.scalar.activation(rstd, inv, Act.Sqrt)
            s_sc = sm.tile([C, 1], f32)
            nc.vector.tensor_tensor(s_sc, rstd, gb[:, 0:1], Alu.mult)
            ms = sm.tile([C, 1], f32)
            nc.vector.tensor_tensor(ms, mean, s_sc, Alu.mult)
            t_sc = sm.tile([C, 1], f32)
            nc.vector.tensor_tensor(t_sc, gb[:, 1:2], ms, Alu.subtract)

            # ---- apply BN (DVE 2x, bf16 out), write out on two queues ----
            for g in range(NGRP):
                lo, hi = g * NW, (g + 1) * NW
                ow = owp.tile([C, NW], bf16, tag="ow")
                nc.vector.tensor_scalar(ow, ypre[:, lo:hi],
                                        s_sc[:, 0:1], t_sc[:, 0:1],
                                        Alu.mult, Alu.add)
                eng = nc.sync if g % 2 == 0 else nc.scalar
                eng.dma_start(out=y_ext[:, lo:hi], in_=ow)

    _split_excess_waits(nc)
    return nc


def _fold_weights(inputs):
    f = np.float32
    W_in = inputs["w_spatial_in"].astype(np.float64)
    W_out = inputs["w_spatial_out"].astype(np.float64)
    dw_sp = inputs["w_dw_spatial"][:, 0, :].astype(np.float64)
    W_proj = inputs["w_out_proj"].astype(np.float64)
    W_mlp2 = inputs["w_mlp2"].astype(np.float64)
    dwt = float(inputs["diff_weight"])

    a_sym = dw_sp[:, 0] + dw_sp[:, 2]
    w1 = dw_sp[:, 1]
    A2 = 0.25 * W_proj @ (W_out * a_sym[None, :]) @ W_in
    B3 = W_proj @ (W_out * w1[None, :]) @ W_in + W_proj
    W_d = 0.25 * dwt * W_proj
    C2 = W_proj @ W_mlp2                     # [c, 32]
    bias_out = W_proj @ inputs["b_mlp2"].astype(np.float64)

    bf = ml_dtypes.bfloat16
    f8 = ml_dtypes.float8_e4m3
    a2t8 = (A2.T * ASC).astype(f8)
    wbf = np.concatenate(
        [B3.T.astype(bf), A2.T.astype(bf), W_d.T.astype(bf),
         np.tile(C2.T.astype(bf), (4, 1))], axis=1)
    wf32 = np.concatenate(
        [inputs["w_ch_out"].astype(f), inputs["w_ch_in"].astype(f),
         inputs["w_mlp1"].T.astype(f),
         inputs["w_ch_dw"][:, 0, :].astype(f),
         np.tile(inputs["b_mlp1"].astype(f), 4)[:, None],
         bias_out.astype(f)[:, None],
         np.stack([inputs["bn_gamma"], inputs["bn_beta"]], 1).astype(f)],
        axis=1)
    return {
        "wbf": np.ascontiguousarray(wbf),
        "a2d": np.ascontiguousarray(np.concatenate([a2t8, a2t8], axis=1)),
        "wf32": np.ascontiguousarray(wf32),
    }


def prepare_in_maps(inputs):
    wmap = _fold_weights(inputs)
    x = np.asarray(inputs["x"]).astype(np.float32)  # [B, C, H, W]
    in_maps = []
    for b in range(NCORES):
        m = dict(wmap)
        xb = x[b].reshape(C, L)
        m["x"] = np.ascontiguousarray(xb.astype(ml_dtypes.bfloat16))
        m["x8"] = np.ascontiguousarray(
            (xb * XSC).astype(ml_dtypes.float8_e4m3))
        in_maps.append(m)
    return in_maps


def kernel(**inputs):
    from concourse.bass_utils import run_bass_kernel_spmd

    inputs = {k: np.asarray(v) for k, v in inputs.items()}
    if "nc" not in _CACHE:
        _CACHE["nc"] = _build_program()
    nc = _CACHE["nc"]

    in_maps = prepare_in_maps(inputs)
    res = run_bass_kernel_spmd(nc, in_maps, list(range(NCORES)))
    out = np.stack([np.asarray(res.results[b]["y"]).astype(np.float32)
                    .reshape(C, Himg, Wimg) for b in range(NCORES)])
    return out


# revision 19
# speedup vs baseline: 1.3693x; 1.0341x over previous
"""OCS fused kernel for Trainium2, data-parallel over batch across 8 cores.

Algebraic restructuring (verified vs reference to ~1e-6 in fp64):

Spatial branch (4 scan orders, shared weights) collapses to a symmetric
5-point stencil with scan-order wrap rules, and the two 1x1 convs fold
through it:  W_proj @ y_sp = A2 @ (4-neighbor sum of x) + (B3 - W_proj) @ x.
The 4-neighbor sums are not materialized: A2 is applied as two fp8
DoubleRow matmuls, each fusing a +-shift pair of x (second K-half read via
a strided AP view), with the A2 magnitude rescaled 2^4 into fp8 range and
x pre-scaled 2^-4 on host (A2 term is ~0.7% of y, fp8 error is negligible
there). Col-scan wraps are two extra small bf16 matmuls.
Channel branch: m = g g^T is rank-1, so the whole conv pipeline collapses
into three [32,128] matmuls on shifted x (weights MP/MQ/MR = u (x) P/Q/R
built on-device from g = sum of x), a silu, and one [128,32] matmul.
Diff branch (large contributor -> bf16): |dx| per axis once (H-axis abs on
DVE, V-axis abs offloaded to GpSimd), pair-sums as shifted adds, W_proj
folds in as W_d @ H + W_d @ V.
BatchNorm: per-core partial (sum, sumsq) -> 1KB AllReduce (warmed up by a
dummy collective at kernel start so the mesh-algo load is off the critical
path) -> affine applied at DVE 2x with bf16 output (host upcasts to f32).
"""

import numpy as np
import ml_dtypes

B, C, Himg, Wimg = 8, 128, 128, 128
L = Himg * Wimg            # 16384
NCORES = 8
NCH = 512                  # matmul chunk columns
NCHUNK = L // NCH          # 32
NPAIR = NCHUNK // 2        # chunk pairs -> 2-bank psum tiles
NW = 2048                  # elementwise window columns (4 chunks)
NGRP = L // NW             # 8
NROW = NW // Wimg          # image rows per window (16)
EPS_BN = 1e-5
NTOT = float(B * L)        # batchnorm population per channel
XSC = 2.0 ** -4            # host prescale of the fp8 x copy
ASC = 2.0 ** 4             # fp8 A2 weight upscale (cancels XSC)

_CACHE = {}


def _make_patched_tc():
    """TileContext whose exit drain splits sem waits one-per-Drain.

    The walrus build in this container rejects Drain instructions carrying
    more than one sem wait ("Too many sync wait commands"). Stock
    TileContext attaches the whole global vector clock to a single tail
    Drain; emit one Drain per outstanding proc instead.
    """
    import bass_rust
    import concourse.tile as tile
    from concourse.vector_clock import ScopedClock

    class PatchedTC(tile.TileContext):
        def _drain_and_barrier(self, tick_clock, wait_clock):
            gc = list(tick_clock.global_clock)
            for i, v in enumerate(gc):
                if v:
                    single = [0] * len(gc)
                    single[i] = v
                    d = self.nc.sync.drain()
                    wait_clock.add_sem_waits(
                        d.ins, ScopedClock({None: bass_rust.VectorClock(single)})
                    )
            self.nc.all_engine_barrier()
            assert self.sems is not None
            popped = self.nc._tile_sem_poison_stack.pop()
            assert popped is self._sem_poison
            self.nc.clear_and_free_semaphores(list(self.sems.allocated().values()))
            self.nc.all_engine_barrier()

    return PatchedTC


def _split_excess_waits(nc):
    """Walrus here allows one sem wait per instruction; hoist extras onto
    same-engine NoOps inserted immediately before the instruction."""
    import bass_rust

    nid = 0
    for blk in nc.main_func.blocks:
        out = []
        for ins in blk.instructions:
            si = getattr(ins, "sync_info", None)
            waits = list(si.on_wait) if si is not None else []
            if len(waits) > 1:
                for w in waits[:-1]:
                    nid += 1
                    nop = bass_rust.InstNoOp(
                        name=f"I-waitsplit-{nid}", ins=[], outs=[])
                    nop.engine = ins.engine
                    nop.sync_info = bass_rust.SyncInfo(
                        on_wait=[w], on_update=[])
                    nc.register_instruction(nop, overwrite=True)
                    out.append(nop)
                si.on_wait = [waits[-1]]
                ins.sync_info = si
            out.append(ins)
        blk.instructions = out


def _build_program():
    import concourse.bass as bass
    import concourse.mybir as mybir

    PatchedTC = _make_patched_tc()

    f32 = mybir.dt.float32
    bf16 = mybir.dt.bfloat16
    fp8 = mybir.dt.float8e4
    u16 = mybir.dt.uint16
    Alu = mybir.AluOpType
    Act = mybir.ActivationFunctionType
    X = mybir.AxisListType.X
    DR = mybir.MatmulPerfMode.DoubleRow

    nc = bass.Bass(target_bir_lowering=False, num_devices=NCORES)

    x_ext = nc.declare_dram_parameter("x", [C, L], bf16, isOutput=False)
    # packed weights: one DMA per dtype class
    wbf_ext = nc.declare_dram_parameter("wbf", [C, 4 * C], bf16,
                                        isOutput=False)
    a2d_ext = nc.declare_dram_parameter("a2d", [C, 2 * C], fp8, isOutput=False)
    wf32_ext = nc.declare_dram_parameter("wf32", [C, 2 * C + 39], f32,
                                         isOutput=False)
    y_ext = nc.declare_dram_parameter("y", [C, L], bf16, isOutput=True)

    with PatchedTC(nc) as tc:
        with (
            tc.tile_pool(name="wp", bufs=1) as wp,
            tc.tile_pool(name="big", bufs=1) as big,
            tc.tile_pool(name="win", bufs=3) as win,
            tc.tile_pool(name="sm", bufs=1) as sm,
            tc.tile_pool(name="dump", bufs=2) as dump,
            tc.tile_pool(name="ow", bufs=4) as owp,
            tc.tile_pool(name="yps", bufs=3, space="PSUM") as yps,
            tc.tile_pool(name="hps", bufs=1, space="PSUM") as hps,
            tc.tile_pool(name="sps", bufs=1, space="PSUM") as sps,
            tc.tile_pool(name="dram", bufs=1, space="DRAM") as dram,
        ):
            # ---- weights to SBUF (gpsimd queue; x on scalar/sync queues) ----
            wbf = wp.tile([C, 4 * C], bf16)
            a2d = wp.tile([C, 2 * C], fp8)
            wf32 = wp.tile([C, 2 * C + 39], f32)
            wb3t = wbf[:, 0:C]
            wa2t = wbf[:, C:2 * C]
            wdt = wbf[:, 2 * C:3 * C]
            c2t4 = wbf[:, 3 * C:4 * C]
            wcho = wf32[:, 0:C]
            wchi = wf32[:, C:2 * C]
            wm1t = wf32[:, 2 * C:2 * C + 32]
            taps = wf32[:, 2 * C + 32:2 * C + 35]
            b1t = wf32[:, 2 * C + 35:2 * C + 36]
            bout = wf32[:, 2 * C + 36:2 * C + 37]
            gb = wf32[:, 2 * C + 37:2 * C + 39]
            ones_row = wp.tile([1, C], f32)
            nc.vector.memset(ones_row, 1.0)
            for t, e in [(wbf, wbf_ext), (a2d, a2d_ext), (wf32, wf32_ext)]:
                nc.gpsimd.dma_start(out=t, in_=e[:])

            # warmup collective: loads the CC mesh algo while compute runs,
            # so the real stats AllReduce skips the ~11us startup
            ccw_in = dram.tile([C, 2], f32)
            ccw_out = dram.tile([C, 2], f32)
            nc.gpsimd.dma_start(out=ccw_in[:], in_=gb)
            nc.gpsimd.collective_compute(
                "AllReduce", Alu.add,
                replica_groups=[list(range(NCORES))],
                ins=[ccw_in.opt()], outs=[ccw_out.opt()])

            # ---- big SBUF arrays ----
            xbf = big.tile([C, L], bf16)     # x (bf16, cast on host)
            x8 = big.tile([C, L], fp8)       # x * 2^-4 (fp8, cast on host)
            ypre = big.tile([C, L], bf16)    # pre-BN output
            h1sb = big.tile([C, NGRP * NCH], bf16)  # silu(h1) packed 4ch/grp

            gsums = sm.tile([C, NGRP], f32)
            ysum = sm.tile([C, NPAIR], f32)
            ysq = sm.tile([C, NPAIR], f32)

            # ---- load x split across two DMA queues (window 7 early:
            #      chunk 0's wrap matmul reads it) ----
            for g in [0, 7, 1, 6]:
                lo, hi = g * NW, (g + 1) * NW
                nc.scalar.dma_start(out=xbf[:, lo:hi], in_=x_ext[:, lo:hi])
            for g in [2, 3, 4, 5]:
                lo, hi = g * NW, (g + 1) * NW
                nc.sync.dma_start(out=xbf[:, lo:hi], in_=x_ext[:, lo:hi])
            # preload the ACT function tables while the x transfers run
            tdum = sm.tile([1, 1], f32)
            for fn in (Act.Copy, Act.Identity, Act.Square, Act.Silu,
                       Act.Sqrt):
                nc.scalar.activation(tdum, ones_row[0:1, 0:1], fn)
            # per-window ACT pass: casts x to the fp8 copy (x8 = x * 2^-4)
            # and accumulates the row sums (x 2^-4, which the rank-1
            # channel-gate algebra cancels) in the same instruction
            for g in [0, 7, 1, 2, 3, 4, 5, 6]:
                lo, hi = g * NW, (g + 1) * NW
                nc.scalar.activation(x8[:, lo:hi], xbf[:, lo:hi], Act.Copy,
                                     scale=XSC,
                                     accum_out=gsums[:, g:g + 1])

            # ---- channel-branch small chain (needs all of x) ----
            gsum = sm.tile([C, 1], f32)
            nc.vector.tensor_reduce(gsum, gsums, X, Alu.add)
            ss_ps = sps.tile([1, 1], f32, tag="sp")
            nc.tensor.matmul(ss_ps, gsum, gsum, start=True, stop=True)
            ss = sm.tile([1, 1], f32)
            nc.vector.tensor_copy(ss, ss_ps)
            rn2 = sm.tile([1, 1], f32)
            nc.vector.reciprocal(rn2, ss)          # 1 / ||gsum||^2

            v_ps = sps.tile([C, 1], f32, tag="sp")
            nc.tensor.matmul(v_ps, wcho, gsum, start=True, stop=True)
            v_sb = sm.tile([C, 1], f32)
            nc.vector.tensor_copy(v_sb, v_ps)
            pqr = sm.tile([C, 3], f32)
            for j in range(3):
                nc.vector.tensor_tensor(pqr[:, j:j + 1], v_sb, taps[:, j:j + 1],
                                        Alu.mult)
            pqr2_ps = sps.tile([C, 3], f32, tag="sp")
            nc.tensor.matmul(pqr2_ps, wchi, pqr, start=True, stop=True)
            pqr2 = sm.tile([C, 3], f32)
            nc.vector.tensor_copy(pqr2, pqr2_ps)

            u_ps = sps.tile([1, 32], f32, tag="sp")
            nc.tensor.matmul(u_ps, gsum, wm1t, start=True, stop=True)
            u_sb = sm.tile([1, 32], f32)
            nc.vector.tensor_copy(u_sb, u_ps)
            u_sc = sm.tile([1, 32], f32)
            nc.vector.tensor_scalar(u_sc, u_sb, rn2[0:1, 0:1], None, Alu.mult)
            # broadcast [1,32] -> [C,32] with a K=1 ones matmul (no DRAM trip)
            ub_ps = sps.tile([C, 32], f32, tag="sp")
            nc.tensor.matmul(ub_ps, ones_row, u_sc, start=True, stop=True)
            u_bc = sm.tile([C, 32], f32)
            nc.vector.tensor_copy(u_bc, ub_ps)

            mqt = sm.tile([C, 32], bf16)
            mpt = sm.tile([C, 32], bf16)
            mrt = sm.tile([C, 32], bf16)
            for t, j in [(mpt, 0), (mqt, 1), (mrt, 2)]:
                nc.vector.tensor_scalar(t, u_bc, pqr2[:, j:j + 1], None,
                                        Alu.mult)

            # ---- streaming main loop ----
            def h1_group(k):
                h1ps = hps.tile([C, NCH], f32)
                for wgt, shift in [(mqt, 0), (mpt, -1), (mrt, +1)]:
                    for j in range(4):
                        n = 4 * k + j
                        n0 = n * NCH
                        lo = n0 + shift
                        hi = n0 + NCH + shift
                        plo, phi = 0, NCH
                        if lo < 0:
                            plo, lo = 1, 0
                        if hi > L:
                            phi, hi = NCH - 1, L
                        nc.tensor.matmul(
                            h1ps[32 * j:32 * j + 32, plo:phi],
                            wgt[:, 0:32], xbf[:, lo:hi],
                            start=(shift == 0), stop=(shift == 1),
                            tile_position=(0, 32 * j))
                nc.scalar.activation(h1sb[:, k * NCH:(k + 1) * NCH], h1ps,
                                     Act.Silu, bias=b1t[:, 0:1])

            def dr_pair(base, istride, n):
                """fp8 ifmap AP reading, for each of n cols j, the K-half pair
                (x8[base+j], x8[base+istride+j]) for a DoubleRow matmul."""
                anchor = x8[:, base:base + 1]
                o = anchor.opt()
                return bass.AP(tensor=anchor.tensor, offset=o.offset,
                               ap=[list(o.ap[0]), [istride, 2], [1, n]])

            a2w = a2d.rearrange("p (i m) -> p i m", i=2)
            a2s = a2d[:, 0:C]   # single (non-DR) fp8 A2 view

            def window_arrays(g):
                """Window arrays: H, V of the diff branch (V-axis abs on
                the otherwise-idle GpSimd engine)."""
                G0 = g * NW
                dh = win.tile([C, NW + 1], bf16, tag="dh")
                H = win.tile([C, NW], bf16, tag="H")
                dv = win.tile([C, NW + 128], bf16, tag="dv")
                V = win.tile([C, NW], bf16, tag="V")

                # dh[j] = x[G0+j] - x[G0+j-1], j in [a, e); |.| in place
                a = 1 if g == 0 else 0
                e = NW if g == NGRP - 1 else NW + 1
                nc.vector.tensor_tensor(dh[:, a:e],
                                        xbf[:, G0 + a:G0 + e],
                                        xbf[:, G0 + a - 1:G0 + e - 1],
                                        Alu.subtract)
                dh2 = dh[:, 0:NW].rearrange("p (r c) -> p r c", c=Wimg)
                nc.vector.memset(dh2[:, :, 0:1], 0.0)   # no cross-row diffs
                nc.vector.memset(dh[:, NW:NW + 1], 0.0)
                dhu = dh.bitcast(u16)
                nc.vector.tensor_scalar(dhu[:, a:e], dhu[:, a:e], 0x7FFF,
                                        None, Alu.bitwise_and)
                # H[j] = |dh[j]| + |dh[j+1]|
                nc.vector.tensor_tensor(H, dh[:, 0:NW], dh[:, 1:NW + 1],
                                        Alu.add)
                # edge fix: col0 += |dh[row,1]| ; col127 += |dh[row,127]|
                H2 = H.rearrange("p (r c) -> p r c", c=Wimg)
                nc.vector.tensor_tensor(H2[:, :, 0:1], H2[:, :, 0:1],
                                        dh2[:, :, 1:2], Alu.add)
                nc.vector.tensor_tensor(H2[:, :, Wimg - 1:Wimg],
                                        H2[:, :, Wimg - 1:Wimg],
                                        dh2[:, :, Wimg - 1:Wimg], Alu.add)

                # dv[j] = x[G0+j] - x[G0+j-128]; abs on GpSimd
                av = 128 if g == 0 else 0
                ev = NW if g == NGRP - 1 else NW + 128
                nc.vector.tensor_tensor(dv[:, av:ev], xbf[:, G0 + av:G0 + ev],
                                        xbf[:, G0 + av - 128:G0 + ev - 128],
                                        Alu.subtract)
                dvu = dv.bitcast(u16)
                nc.vector.tensor_scalar(dvu[:, av:ev], dvu[:, av:ev], 0x7FFF,
                                        None, Alu.bitwise_and)
                if g == 0:
                    nc.vector.memset(dv[:, 0:128], 0.0)   # row 0: no up-diff
                if g == NGRP - 1:
                    # last row reflect: pair partner := own value -> 2|dv|
                    nc.vector.tensor_copy(dv[:, NW:NW + 128],
                                          dv[:, NW - 128:NW])
                # V[j] = |dv[j]| + |dv[j+128]|
                nc.vector.tensor_tensor(V, dv[:, 0:NW], dv[:, 128:NW + 128],
                                        Alu.add)
                if g == 0:
                    # row 0 reflect: V = 2*|dv[j+128]|
                    nc.vector.tensor_tensor(V[:, 0:128], V[:, 0:128],
                                            dv[:, 128:256], Alu.add)
                return H, V

            def chunk_pair(m, H, V):
                """Chunks 2m, 2m+1 into one 2-bank psum tile; the bf16
                matmuls (B3, Wd@H, Wd@V) run 1024 wide."""
                ps = yps.tile([C, 2 * NCH], f32)
                NP = 2 * NCH
                p0 = 2 * m * NCH          # first flat column of the pair
                off = (2 * m % 4) * NCH   # H/V window-local offset
                for h in range(2):
                    nc.tensor.matmul(ps[:, h * NCH:(h + 1) * NCH], wb3t,
                                     xbf[:, p0 + h * NCH:p0 + (h + 1) * NCH],
                                     start=True, stop=False)
                for h in range(2):
                    n = 2 * m + h
                    n0 = n * NCH
                    q = h * NCH
                    # A2 @ (x[l-1]+x[l+1]) as one fp8 DoubleRow matmul
                    plo = 1 if n == 0 else 0
                    phi = NCH - 1 if n == NCHUNK - 1 else NCH
                    nc.tensor.matmul(ps[:, q + plo:q + phi], a2w,
                                     dr_pair(n0 + plo - 1, 2, phi - plo),
                                     start=False, stop=False, perf_mode=DR)
                    if n == 0:      # l=0 keeps only the right neighbor
                        nc.tensor.matmul(ps[:, q:q + 1], a2s, x8[:, 1:2],
                                         start=False, stop=False)
                    if n == NCHUNK - 1:   # l=L-1 keeps only the left
                        nc.tensor.matmul(ps[:, q + NCH - 1:q + NCH], a2s,
                                         x8[:, L - 2:L - 1],
                                         start=False, stop=False)
                    # A2 @ (x[l-128]+x[l+128]) as one fp8 DoubleRow matmul
                    vlo = 128 if n == 0 else 0
                    vhi = NCH - 128 if n == NCHUNK - 1 else NCH
                    nc.tensor.matmul(ps[:, q + vlo:q + vhi], a2w,
                                     dr_pair(n0 + vlo - 128, 256, vhi - vlo),
                                     start=False, stop=False, perf_mode=DR)
                    if n == 0:      # first image row keeps only down
                        nc.tensor.matmul(ps[:, q:q + 128], a2s,
                                         x8[:, 128:256],
                                         start=False, stop=False)
                    if n == NCHUNK - 1:   # last image row keeps only up
                        nc.tensor.matmul(ps[:, q + NCH - 128:q + NCH], a2s,
                                         x8[:, L - 256:L - 128],
                                         start=False, stop=False)
                    if n == 0:
                        # col-scan wrap: l=j gets x[(h-1)w + j - 1]
                        nc.tensor.matmul(ps[:, q + 1:q + 128], wa2t,
                                         xbf[:, L - Wimg:L - 1],
                                         start=False, stop=False)
                    if n == NCHUNK - 1:
                        # col-scan wrap: l=(h-1)w+j gets x[j+1]
                        nc.tensor.matmul(ps[:, q + NCH - 128:q + NCH - 1],
                                         wa2t, xbf[:, 1:128],
                                         start=False, stop=False)
                # diff branch
                for h in range(2):
                    pso = ps[:, h * NCH:(h + 1) * NCH]
                    o2 = off + h * NCH
                    nc.tensor.matmul(pso, wdt, H[:, o2:o2 + NCH],
                                     start=False, stop=False)
                    nc.tensor.matmul(pso, wdt, V[:, o2:o2 + NCH],
                                     start=False, stop=False)
                # channel contribution (row-tiled, K=32)
                for h in range(2):
                    n = 2 * m + h
                    j = n % 4
                    nc.tensor.matmul(ps[:, h * NCH:h * NCH + NCH],
                                     c2t4[32 * j:32 * j + 32, :],
                                     h1sb[32 * j:32 * j + 32,
                                          (n // 4) * NCH:(n // 4 + 1) * NCH],
                                     start=False, stop=(h == 1),
                                     tile_position=(32 * j, 0))
                nc.scalar.activation(ypre[:, p0:p0 + NP], ps,
                                     Act.Identity, bias=bout[:, 0:1],
                                     accum_out=ysum[:, m:m + 1])
                dmp = dump.tile([C, NP], bf16, tag="sq")
                nc.scalar.activation(dmp, ps, Act.Square,
                                     accum_out=ysq[:, m:m + 1])

            h1_group(0)
            win_arrays = window_arrays(0)
            for k in range(NGRP):
                nxt = None
                if k + 1 < NGRP:
                    h1_group(k + 1)
                    nxt = window_arrays(k + 1)
                chunk_pair(2 * k, *win_arrays)
                chunk_pair(2 * k + 1, *win_arrays)
                win_arrays = nxt

            # ---- global BN stats via AllReduce ----
            # ysq tracked z = y - bout (PSUM, pre-bias):
            #   sum(y^2) = sum(z^2) + 2*bout*sum(y) - L*bout^2
            stats = sm.tile([C, 2], f32)
            nc.vector.tensor_reduce(stats[:, 0:1], ysum, X, Alu.add)
            nc.vector.tensor_reduce(stats[:, 1:2], ysq, X, Alu.add)
            cb = sm.tile([C, 1], f32)
            nc.vector.tensor_tensor(cb, bout, stats[:, 0:1], Alu.mult)
            nc.vector.scalar_tensor_tensor(stats[:, 1:2], cb, 2.0,
                                           stats[:, 1:2], Alu.mult, Alu.add)
            bsq = sm.tile([C, 1], f32)
            nc.vector.tensor_tensor(bsq, bout, bout, Alu.mult)
            nc.vector.scalar_tensor_tensor(stats[:, 1:2], bsq, -float(L),
                                           stats[:, 1:2], Alu.mult, Alu.add)
            # prefetch the sqrt ACT table while the collective runs
            sqpre = sm.tile([C, 1], f32)
            nc.scalar.activation(sqpre, stats[:, 1:2], Act.Sqrt)
            cc_in = dram.tile([C, 2], f32)
            cc_out = dram.tile([C, 2], f32)
            nc.gpsimd.dma_start(out=cc_in[:], in_=stats)
            nc.gpsimd.collective_compute(
                "AllReduce", Alu.add,
                replica_groups=[list(range(NCORES))],
                ins=[cc_in.opt()], outs=[cc_out.opt()])
            statsr = sm.tile([C, 2], f32)
            nc.gpsimd.dma_start(out=statsr, in_=cc_out[:])

            mean = sm.tile([C, 1], f32)
            ex2 = sm.tile([C, 1], f32)
            nc.vector.tensor_scalar(mean, statsr[:, 0:1], 1.0 / NTOT, None,
                                    Alu.mult)
            nc.vector.tensor_scalar(ex2, statsr[:, 1:2], 1.0 / NTOT, None,
                                    Alu.mult)
            m2 = sm.tile([C, 1], f32)
            nc.vector.tensor_tensor(m2, mean, mean, Alu.mult)
            varep = sm.tile([C, 1], f32)
            nc.vector.tensor_tensor(varep, ex2, m2, Alu.subtract)
            nc.vector.tensor_scalar(varep, varep, EPS_BN, None, Alu.add)
            inv = sm.tile([C, 1], f32)
            nc.vector.reciprocal(inv, varep)
            rstd = sm.tile([C, 1], f32)
            nc.scalar.activation(rstd, inv, Act.Sqrt)
            s_sc = sm.tile([C, 1], f32)
            nc.vector.tensor_tensor(s_sc, rstd, gb[:, 0:1], Alu.mult)
            ms = sm.tile([C, 1], f32)
            nc.vector.tensor_tensor(ms, mean, s_sc, Alu.mult)
            t_sc = sm.tile([C, 1], f32)
            nc.vector.tensor_tensor(t_sc, gb[:, 1:2], ms, Alu.subtract)

            # ---- apply BN (DVE 2x, bf16 out), write out on two queues ----
            for g in range(NGRP):
                lo, hi = g * NW, (g + 1) * NW
                ow = owp.tile([C, NW], bf16, tag="ow")
                nc.vector.tensor_scalar(ow, ypre[:, lo:hi],
                                        s_sc[:, 0:1], t_sc[:, 0:1],
                                        Alu.mult, Alu.add)
                eng = nc.sync if g % 2 == 0 else nc.scalar
                eng.dma_start(out=y_ext[:, lo:hi], in_=ow)

    _split_excess_waits(nc)
    return nc


def _fold_weights(inputs):
    f = np.float32
    W_in = inputs["w_spatial_in"].astype(np.float64)
    W_out = inputs["w_spatial_out"].astype(np.float64)
    dw_sp = inputs["w_dw_spatial"][:, 0, :].astype(np.float64)
    W_proj = inputs["w_out_proj"].astype(np.float64)
    W_mlp2 = inputs["w_mlp2"].astype(np.float64)
    dwt = float(inputs["diff_weight"])

    a_sym = dw_sp[:, 0] + dw_sp[:, 2]
    w1 = dw_sp[:, 1]
    A2 = 0.25 * W_proj @ (W_out * a_sym[None, :]) @ W_in
    B3 = W_proj @ (W_out * w1[None, :]) @ W_in + W_proj
    W_d = 0.25 * dwt * W_proj
    C2 = W_proj @ W_mlp2                     # [c, 32]
    bias_out = W_proj @ inputs["b_mlp2"].astype(np.float64)

    bf = ml_dtypes.bfloat16
    f8 = ml_dtypes.float8_e4m3
    a2t8 = (A2.T * ASC).astype(f8)
    wbf = np.concatenate(
        [B3.T.astype(bf), A2.T.astype(bf), W_d.T.astype(bf),
         np.tile(C2.T.astype(bf), (4, 1))], axis=1)
    wf32 = np.concatenate(
        [inputs["w_ch_out"].astype(f), inputs["w_ch_in"].astype(f),
         inputs["w_mlp1"].T.astype(f),
         inputs["w_ch_dw"][:, 0, :].astype(f),
         np.tile(inputs["b_mlp1"].astype(f), 4)[:, None],
         bias_out.astype(f)[:, None],
         np.stack([inputs["bn_gamma"], inputs["bn_beta"]], 1).astype(f)],
        axis=1)
    return {
        "wbf": np.ascontiguousarray(wbf),
        "a2d": np.ascontiguousarray(np.concatenate([a2t8, a2t8], axis=1)),
        "wf32": np.ascontiguousarray(wf32),
    }


def prepare_in_maps(inputs):
    wmap = _fold_weights(inputs)
    x = np.asarray(inputs["x"]).astype(np.float32)  # [B, C, H, W]
    in_maps = []
    for b in range(NCORES):
        m = dict(wmap)
        xb = x[b].reshape(C, L)
        m["x"] = np.ascontiguousarray(xb.astype(ml_dtypes.bfloat16))
        in_maps.append(m)
    return in_maps


def kernel(**inputs):
    from concourse.bass_utils import run_bass_kernel_spmd

    inputs = {k: np.asarray(v) for k, v in inputs.items()}
    if "nc" not in _CACHE:
        _CACHE["nc"] = _build_program()
    nc = _CACHE["nc"]

    in_maps = prepare_in_maps(inputs)
    res = run_bass_kernel_spmd(nc, in_maps, list(range(NCORES)))
    out = np.stack([np.asarray(res.results[b]["y"]).astype(np.float32)
                    .reshape(C, Himg, Wimg) for b in range(NCORES)])
    return out


# revision 21
# speedup vs baseline: 1.4377x; 1.0499x over previous
"""OCS fused kernel for Trainium2, data-parallel over batch across 8 cores.

Algebraic restructuring (verified vs reference to ~1e-6 in fp64):

Spatial branch (4 scan orders, shared weights) collapses to a symmetric
5-point stencil with scan-order wrap rules, and the two 1x1 convs fold
through it:  W_proj @ y_sp = A2 @ (4-neighbor sum of x) + (B3 - W_proj) @ x.
The 4-neighbor sums are not materialized: A2 is applied as two fp8
DoubleRow matmuls, each fusing a +-shift pair of x (second K-half read via
a strided AP view), with the A2 magnitude rescaled 2^4 into fp8 range and
x pre-scaled 2^-4 on host (A2 term is ~0.7% of y, fp8 error is negligible
there). Col-scan wraps are two extra small bf16 matmuls.
Channel branch: m = g g^T is rank-1, so the whole conv pipeline collapses
into three [32,128] matmuls on shifted x (weights MP/MQ/MR = u (x) P/Q/R
built on-device from g = sum of x), a silu, and one [128,32] matmul.
Diff branch (large contributor -> bf16): |dx| per axis once (H-axis abs on
DVE, V-axis abs offloaded to GpSimd), pair-sums as shifted adds, W_proj
folds in as W_d @ H + W_d @ V.
BatchNorm: per-core partial (sum, sumsq) -> 1KB AllReduce (warmed up by a
dummy collective at kernel start so the mesh-algo load is off the critical
path) -> affine applied at DVE 2x with bf16 output (host upcasts to f32).
"""

import numpy as np
import ml_dtypes

B, C, Himg, Wimg = 8, 128, 128, 128
L = Himg * Wimg            # 16384
NCORES = 8
NCH = 512                  # matmul chunk columns
NCHUNK = L // NCH          # 32
NPAIR = NCHUNK // 2        # chunk pairs -> 2-bank psum tiles
NW = 2048                  # elementwise window columns (4 chunks)
NGRP = L // NW             # 8
NROW = NW // Wimg          # image rows per window (16)
EPS_BN = 1e-5
NTOT = float(B * L)        # batchnorm population per channel
XSC = 2.0 ** -4            # host prescale of the fp8 x copy
ASC = 2.0 ** 4             # fp8 A2 weight upscale (cancels XSC)

_CACHE = {}


def _make_patched_tc():
    """TileContext whose exit drain splits sem waits one-per-Drain.

    The walrus build in this container rejects Drain instructions carrying
    more than one sem wait ("Too many sync wait commands"). Stock
    TileContext attaches the whole global vector clock to a single tail
    Drain; emit one Drain per outstanding proc instead.
    """
    import bass_rust
    import concourse.tile as tile
    from concourse.vector_clock import ScopedClock

    class PatchedTC(tile.TileContext):
        def _drain_and_barrier(self, tick_clock, wait_clock):
            gc = list(tick_clock.global_clock)
            for i, v in enumerate(gc):
                if v:
                    single = [0] * len(gc)
                    single[i] = v
                    d = self.nc.sync.drain()
                    wait_clock.add_sem_waits(
                        d.ins, ScopedClock({None: bass_rust.VectorClock(single)})
                    )
            self.nc.all_engine_barrier()
            assert self.sems is not None
            popped = self.nc._tile_sem_poison_stack.pop()
            assert popped is self._sem_poison
            self.nc.clear_and_free_semaphores(list(self.sems.allocated().values()))
            self.nc.all_engine_barrier()

    return PatchedTC


def _split_excess_waits(nc):
    """Walrus here allows one sem wait per instruction; hoist extras onto
    same-engine NoOps inserted immediately before the instruction."""
    import bass_rust

    nid = 0
    for blk in nc.main_func.blocks:
        out = []
        for ins in blk.instructions:
            si = getattr(ins, "sync_info", None)
            waits = list(si.on_wait) if si is not None else []
            if len(waits) > 1:
                for w in waits[:-1]:
                    nid += 1
                    nop = bass_rust.InstNoOp(
                        name=f"I-waitsplit-{nid}", ins=[], outs=[])
                    nop.engine = ins.engine
                    nop.sync_info = bass_rust.SyncInfo(
                        on_wait=[w], on_update=[])
                    nc.register_instruction(nop, overwrite=True)
                    out.append(nop)
                si.on_wait = [waits[-1]]
                ins.sync_info = si
            out.append(ins)
        blk.instructions = out


def _build_program():
    import concourse.bass as bass
    import concourse.mybir as mybir

    PatchedTC = _make_patched_tc()

    f32 = mybir.dt.float32
    bf16 = mybir.dt.bfloat16
    fp8 = mybir.dt.float8e4
    u16 = mybir.dt.uint16
    Alu = mybir.AluOpType
    Act = mybir.ActivationFunctionType
    X = mybir.AxisListType.X
    DR = mybir.MatmulPerfMode.DoubleRow

    nc = bass.Bass(target_bir_lowering=False, num_devices=NCORES)

    x_ext = nc.declare_dram_parameter("x", [C, L], bf16, isOutput=False)
    # packed weights: one DMA per dtype class
    wbf_ext = nc.declare_dram_parameter("wbf", [C, 4 * C], bf16,
                                        isOutput=False)
    a2d_ext = nc.declare_dram_parameter("a2d", [C, 2 * C], fp8, isOutput=False)
    wf32_ext = nc.declare_dram_parameter("wf32", [C, 2 * C + 39], f32,
                                         isOutput=False)
    y_ext = nc.declare_dram_parameter("y", [C, L], bf16, isOutput=True)

    with PatchedTC(nc) as tc:
        with (
            tc.tile_pool(name="wp", bufs=1) as wp,
            tc.tile_pool(name="big", bufs=1) as big,
            tc.tile_pool(name="win", bufs=3) as win,
            tc.tile_pool(name="sm", bufs=1) as sm,
            tc.tile_pool(name="dump", bufs=2) as dump,
            tc.tile_pool(name="ow", bufs=4) as owp,
            tc.tile_pool(name="yps", bufs=3, space="PSUM") as yps,
            tc.tile_pool(name="hps", bufs=1, space="PSUM") as hps,
            tc.tile_pool(name="sps", bufs=1, space="PSUM") as sps,
            tc.tile_pool(name="dram", bufs=1, space="DRAM") as dram,
        ):
            # ---- weights to SBUF (gpsimd queue; x on scalar/sync queues) ----
            wbf = wp.tile([C, 4 * C], bf16)
            a2d = wp.tile([C, 2 * C], fp8)
            wf32 = wp.tile([C, 2 * C + 39], f32)
            wb3t = wbf[:, 0:C]
            wa2t = wbf[:, C:2 * C]
            wdt = wbf[:, 2 * C:3 * C]
            c2t4 = wbf[:, 3 * C:4 * C]
            wcho = wf32[:, 0:C]
            wchi = wf32[:, C:2 * C]
            wm1t = wf32[:, 2 * C:2 * C + 32]
            taps = wf32[:, 2 * C + 32:2 * C + 35]
            b1t = wf32[:, 2 * C + 35:2 * C + 36]
            bout = wf32[:, 2 * C + 36:2 * C + 37]
            gb = wf32[:, 2 * C + 37:2 * C + 39]
            ones_row = wp.tile([1, C], f32)
            nc.vector.memset(ones_row, 1.0)
            for t, e in [(wbf, wbf_ext), (a2d, a2d_ext), (wf32, wf32_ext)]:
                nc.gpsimd.dma_start(out=t, in_=e[:])

            # warmup collective: loads the CC mesh algo while compute runs,
            # so the real stats AllReduce skips the ~11us startup
            ccw_in = dram.tile([C, 2], f32)
            ccw_out = dram.tile([C, 2], f32)
            nc.gpsimd.dma_start(out=ccw_in[:], in_=gb)
            nc.gpsimd.collective_compute(
                "AllReduce", Alu.add,
                replica_groups=[list(range(NCORES))],
                ins=[ccw_in.opt()], outs=[ccw_out.opt()])

            # ---- big SBUF arrays ----
            xbf = big.tile([C, L], bf16)     # x (bf16, cast on host)
            x8 = big.tile([C, L], fp8)       # x * 2^-4 (fp8, cast on host)
            ypre = big.tile([C, L], bf16)    # pre-BN output
            h1sb = big.tile([C, NGRP * NCH], bf16)  # silu(h1) packed 4ch/grp

            gsums = sm.tile([C, NGRP], f32)
            ysum = sm.tile([C, NPAIR], f32)
            ysq = sm.tile([C, NPAIR], f32)

            # ---- load x split across two DMA queues (window 7 early:
            #      chunk 0's wrap matmul reads it) ----
            for g in [0, 7, 1, 6]:
                lo, hi = g * NW, (g + 1) * NW
                nc.scalar.dma_start(out=xbf[:, lo:hi], in_=x_ext[:, lo:hi])
            for g in [2, 3, 4, 5]:
                lo, hi = g * NW, (g + 1) * NW
                nc.sync.dma_start(out=xbf[:, lo:hi], in_=x_ext[:, lo:hi])
            # preload the ACT function tables while the x transfers run
            tdum = sm.tile([1, 1], f32)
            for fn in (Act.Copy, Act.Identity, Act.Square, Act.Silu,
                       Act.Sqrt):
                nc.scalar.activation(tdum, ones_row[0:1, 0:1], fn)
            # row sums (scaled 2^-4, which the rank-1 channel-gate algebra
            # cancels): 4 windows ride the fp8-cast ACT pass, 4 reduce on
            # DVE, so gsum -> h1-weights resolves as soon as x lands
            for g in [0, 7, 1, 6]:
                lo, hi = g * NW, (g + 1) * NW
                nc.scalar.activation(x8[:, lo:hi], xbf[:, lo:hi], Act.Copy,
                                     scale=XSC,
                                     accum_out=gsums[:, g:g + 1])
            for g in [2, 3, 4, 5]:
                lo, hi = g * NW, (g + 1) * NW
                nc.vector.tensor_reduce(gsums[:, g:g + 1], xbf[:, lo:hi],
                                        X, Alu.add)
            # match the ACT partials' 2^-4 scale
            nc.vector.tensor_scalar(gsums[:, 2:6], gsums[:, 2:6], XSC, None,
                                    Alu.mult)
            # remaining fp8-cast windows (needed only once chunk pair 4
            # runs, well after the gsum gate)
            for g in [2, 3, 4, 5]:
                lo, hi = g * NW, (g + 1) * NW
                nc.scalar.activation(x8[:, lo:hi], xbf[:, lo:hi], Act.Copy,
                                     scale=XSC)

            # ---- channel-branch small chain (needs all of x) ----
            gsum = sm.tile([C, 1], f32)
            nc.vector.tensor_reduce(gsum, gsums, X, Alu.add)
            ss_ps = sps.tile([1, 1], f32, tag="sp")
            nc.tensor.matmul(ss_ps, gsum, gsum, start=True, stop=True)
            ss = sm.tile([1, 1], f32)
            nc.vector.tensor_copy(ss, ss_ps)
            rn2 = sm.tile([1, 1], f32)
            nc.vector.reciprocal(rn2, ss)          # 1 / ||gsum||^2

            v_ps = sps.tile([C, 1], f32, tag="sp")
            nc.tensor.matmul(v_ps, wcho, gsum, start=True, stop=True)
            v_sb = sm.tile([C, 1], f32)
            nc.vector.tensor_copy(v_sb, v_ps)
            pqr = sm.tile([C, 3], f32)
            for j in range(3):
                nc.vector.tensor_tensor(pqr[:, j:j + 1], v_sb, taps[:, j:j + 1],
                                        Alu.mult)
            pqr2_ps = sps.tile([C, 3], f32, tag="sp")
            nc.tensor.matmul(pqr2_ps, wchi, pqr, start=True, stop=True)
            pqr2 = sm.tile([C, 3], f32)
            nc.vector.tensor_copy(pqr2, pqr2_ps)

            u_ps = sps.tile([1, 32], f32, tag="sp")
            nc.tensor.matmul(u_ps, gsum, wm1t, start=True, stop=True)
            u_sb = sm.tile([1, 32], f32)
            nc.vector.tensor_copy(u_sb, u_ps)
            u_sc = sm.tile([1, 32], f32)
            nc.vector.tensor_scalar(u_sc, u_sb, rn2[0:1, 0:1], None, Alu.mult)
            # broadcast [1,32] -> [C,32] with a K=1 ones matmul (no DRAM trip)
            ub_ps = sps.tile([C, 32], f32, tag="sp")
            nc.tensor.matmul(ub_ps, ones_row, u_sc, start=True, stop=True)
            u_bc = sm.tile([C, 32], f32)
            nc.vector.tensor_copy(u_bc, ub_ps)

            mqt = sm.tile([C, 32], bf16)
            mpt = sm.tile([C, 32], bf16)
            mrt = sm.tile([C, 32], bf16)
            for t, j in [(mpt, 0), (mqt, 1), (mrt, 2)]:
                nc.vector.tensor_scalar(t, u_bc, pqr2[:, j:j + 1], None,
                                        Alu.mult)

            # ---- streaming main loop ----
            def h1_group(k):
                h1ps = hps.tile([C, NCH], f32)
                for wgt, shift in [(mqt, 0), (mpt, -1), (mrt, +1)]:
                    for j in range(4):
                        n = 4 * k + j
                        n0 = n * NCH
                        lo = n0 + shift
                        hi = n0 + NCH + shift
                        plo, phi = 0, NCH
                        if lo < 0:
                            plo, lo = 1, 0
                        if hi > L:
                            phi, hi = NCH - 1, L
                        nc.tensor.matmul(
                            h1ps[32 * j:32 * j + 32, plo:phi],
                            wgt[:, 0:32], xbf[:, lo:hi],
                            start=(shift == 0), stop=(shift == 1),
                            tile_position=(0, 32 * j))
                nc.scalar.activation(h1sb[:, k * NCH:(k + 1) * NCH], h1ps,
                                     Act.Silu, bias=b1t[:, 0:1])

            def dr_pair(base, istride, n):
                """fp8 ifmap AP reading, for each of n cols j, the K-half pair
                (x8[base+j], x8[base+istride+j]) for a DoubleRow matmul."""
                anchor = x8[:, base:base + 1]
                o = anchor.opt()
                return bass.AP(tensor=anchor.tensor, offset=o.offset,
                               ap=[list(o.ap[0]), [istride, 2], [1, n]])

            a2w = a2d.rearrange("p (i m) -> p i m", i=2)
            a2s = a2d[:, 0:C]   # single (non-DR) fp8 A2 view

            def window_arrays(g):
                """Window arrays: H, V of the diff branch (V-axis abs on
                the otherwise-idle GpSimd engine)."""
                G0 = g * NW
                dh = win.tile([C, NW + 1], bf16, tag="dh")
                H = win.tile([C, NW], bf16, tag="H")
                dv = win.tile([C, NW + 128], bf16, tag="dv")
                V = win.tile([C, NW], bf16, tag="V")

                # dh[j] = x[G0+j] - x[G0+j-1], j in [a, e); |.| in place
                a = 1 if g == 0 else 0
                e = NW if g == NGRP - 1 else NW + 1
                nc.vector.tensor_tensor(dh[:, a:e],
                                        xbf[:, G0 + a:G0 + e],
                                        xbf[:, G0 + a - 1:G0 + e - 1],
                                        Alu.subtract)
                dh2 = dh[:, 0:NW].rearrange("p (r c) -> p r c", c=Wimg)
                nc.vector.memset(dh2[:, :, 0:1], 0.0)   # no cross-row diffs
                nc.vector.memset(dh[:, NW:NW + 1], 0.0)
                dhu = dh.bitcast(u16)
                nc.vector.tensor_scalar(dhu[:, a:e], dhu[:, a:e], 0x7FFF,
                                        None, Alu.bitwise_and)
                # H[j] = |dh[j]| + |dh[j+1]|
                nc.vector.tensor_tensor(H, dh[:, 0:NW], dh[:, 1:NW + 1],
                                        Alu.add)
                # edge fix: col0 += |dh[row,1]| ; col127 += |dh[row,127]|
                H2 = H.rearrange("p (r c) -> p r c", c=Wimg)
                nc.vector.tensor_tensor(H2[:, :, 0:1], H2[:, :, 0:1],
                                        dh2[:, :, 1:2], Alu.add)
                nc.vector.tensor_tensor(H2[:, :, Wimg - 1:Wimg],
                                        H2[:, :, Wimg - 1:Wimg],
                                        dh2[:, :, Wimg - 1:Wimg], Alu.add)

                # dv[j] = x[G0+j] - x[G0+j-128]; abs on GpSimd
                av = 128 if g == 0 else 0
                ev = NW if g == NGRP - 1 else NW + 128
                nc.vector.tensor_tensor(dv[:, av:ev], xbf[:, G0 + av:G0 + ev],
                                        xbf[:, G0 + av - 128:G0 + ev - 128],
                                        Alu.subtract)
                dvu = dv.bitcast(u16)
                nc.vector.tensor_scalar(dvu[:, av:ev], dvu[:, av:ev], 0x7FFF,
                                        None, Alu.bitwise_and)
                if g == 0:
                    nc.vector.memset(dv[:, 0:128], 0.0)   # row 0: no up-diff
                if g == NGRP - 1:
                    # last row reflect: pair partner := own value -> 2|dv|
                    nc.vector.tensor_copy(dv[:, NW:NW + 128],
                                          dv[:, NW - 128:NW])
                # V[j] = |dv[j]| + |dv[j+128]|
                nc.vector.tensor_tensor(V, dv[:, 0:NW], dv[:, 128:NW + 128],
                                        Alu.add)
                if g == 0:
                    # row 0 reflect: V = 2*|dv[j+128]|
                    nc.vector.tensor_tensor(V[:, 0:128], V[:, 0:128],
                                            dv[:, 128:256], Alu.add)
                return H, V

            def chunk_pair(m, H, V):
                """Chunks 2m, 2m+1 into one 2-bank psum tile; the bf16
                matmuls (B3, Wd@H, Wd@V) run 1024 wide."""
                ps = yps.tile([C, 2 * NCH], f32)
                NP = 2 * NCH
                p0 = 2 * m * NCH          # first flat column of the pair
                off = (2 * m % 4) * NCH   # H/V window-local offset
                for h in range(2):
                    nc.tensor.matmul(ps[:, h * NCH:(h + 1) * NCH], wb3t,
                                     xbf[:, p0 + h * NCH:p0 + (h + 1) * NCH],
                                     start=True, stop=False)
                for h in range(2):
                    n = 2 * m + h
                    n0 = n * NCH
                    q = h * NCH
                    # A2 @ (x[l-1]+x[l+1]) as one fp8 DoubleRow matmul
                    plo = 1 if n == 0 else 0
                    phi = NCH - 1 if n == NCHUNK - 1 else NCH
                    nc.tensor.matmul(ps[:, q + plo:q + phi], a2w,
                                     dr_pair(n0 + plo - 1, 2, phi - plo),
                                     start=False, stop=False, perf_mode=DR)
                    if n == 0:      # l=0 keeps only the right neighbor
                        nc.tensor.matmul(ps[:, q:q + 1], a2s, x8[:, 1:2],
                                         start=False, stop=False)
                    if n == NCHUNK - 1:   # l=L-1 keeps only the left
                        nc.tensor.matmul(ps[:, q + NCH - 1:q + NCH], a2s,
                                         x8[:, L - 2:L - 1],
                                         start=False, stop=False)
                    # A2 @ (x[l-128]+x[l+128]) as one fp8 DoubleRow matmul
                    vlo = 128 if n == 0 else 0
                    vhi = NCH - 128 if n == NCHUNK - 1 else NCH
                    nc.tensor.matmul(ps[:, q + vlo:q + vhi], a2w,
                                     dr_pair(n0 + vlo - 128, 256, vhi - vlo),
                                     start=False, stop=False, perf_mode=DR)
                    if n == 0:      # first image row keeps only down
                        nc.tensor.matmul(ps[:, q:q + 128], a2s,
                                         x8[:, 128:256],
                                         start=False, stop=False)
                    if n == NCHUNK - 1:   # last image row keeps only up
                        nc.tensor.matmul(ps[:, q + NCH - 128:q + NCH], a2s,
                                         x8[:, L - 256:L - 128],
                                         start=False, stop=False)
                    if n == 0:
                        # col-scan wrap: l=j gets x[(h-1)w + j - 1]
                        nc.tensor.matmul(ps[:, q + 1:q + 128], wa2t,
                                         xbf[:, L - Wimg:L - 1],
                                         start=False, stop=False)
                    if n == NCHUNK - 1:
                        # col-scan wrap: l=(h-1)w+j gets x[j+1]
                        nc.tensor.matmul(ps[:, q + NCH - 128:q + NCH - 1],
                                         wa2t, xbf[:, 1:128],
                                         start=False, stop=False)
                # diff branch
                for h in range(2):
                    pso = ps[:, h * NCH:(h + 1) * NCH]
                    o2 = off + h * NCH
                    nc.tensor.matmul(pso, wdt, H[:, o2:o2 + NCH],
                                     start=False, stop=False)
                    nc.tensor.matmul(pso, wdt, V[:, o2:o2 + NCH],
                                     start=False, stop=False)
                # channel contribution (row-tiled, K=32)
                for h in range(2):
                    n = 2 * m + h
                    j = n % 4
                    nc.tensor.matmul(ps[:, h * NCH:h * NCH + NCH],
                                     c2t4[32 * j:32 * j + 32, :],
                                     h1sb[32 * j:32 * j + 32,
                                          (n // 4) * NCH:(n // 4 + 1) * NCH],
                                     start=False, stop=(h == 1),
                                     tile_position=(32 * j, 0))
                nc.scalar.activation(ypre[:, p0:p0 + NP], ps,
                                     Act.Identity, bias=bout[:, 0:1],
                                     accum_out=ysum[:, m:m + 1])
                dmp = dump.tile([C, NP], bf16, tag="sq")
                nc.scalar.activation(dmp, ps, Act.Square,
                                     accum_out=ysq[:, m:m + 1])

            h1_group(0)
            win_arrays = window_arrays(0)
            for k in range(NGRP):
                nxt = None
                if k + 1 < NGRP:
                    h1_group(k + 1)
                    nxt = window_arrays(k + 1)
                chunk_pair(2 * k, *win_arrays)
                chunk_pair(2 * k + 1, *win_arrays)
                win_arrays = nxt

            # ---- global BN stats via AllReduce ----
            # ysq tracked z = y - bout (PSUM, pre-bias):
            #   sum(y^2) = sum(z^2) + 2*bout*sum(y) - L*bout^2
            stats = sm.tile([C, 2], f32)
            nc.vector.tensor_reduce(stats[:, 0:1], ysum, X, Alu.add)
            nc.vector.tensor_reduce(stats[:, 1:2], ysq, X, Alu.add)
            cb = sm.tile([C, 1], f32)
            nc.vector.tensor_tensor(cb, bout, stats[:, 0:1], Alu.mult)
            nc.vector.scalar_tensor_tensor(stats[:, 1:2], cb, 2.0,
                                           stats[:, 1:2], Alu.mult, Alu.add)
            bsq = sm.tile([C, 1], f32)
            nc.vector.tensor_tensor(bsq, bout, bout, Alu.mult)
            nc.vector.scalar_tensor_tensor(stats[:, 1:2], bsq, -float(L),
                                           stats[:, 1:2], Alu.mult, Alu.add)
            # prefetch the sqrt ACT table while the collective runs
            sqpre = sm.tile([C, 1], f32)
            nc.scalar.activation(sqpre, stats[:, 1:2], Act.Sqrt)
            cc_in = dram.tile([C, 2], f32)
            cc_out = dram.tile([C, 2], f32)
            nc.gpsimd.dma_start(out=cc_in[:], in_=stats)
            nc.gpsimd.collective_compute(
                "AllReduce", Alu.add,
                replica_groups=[list(range(NCORES))],
                ins=[cc_in.opt()], outs=[cc_out.opt()])
            statsr = sm.tile([C, 2], f32)
            nc.gpsimd.dma_start(out=statsr, in_=cc_out[:])

            mean = sm.tile([C, 1], f32)
            ex2 = sm.tile([C, 1], f32)
            nc.vector.tensor_scalar(mean, statsr[:, 0:1], 1.0 / NTOT, None,
                                    Alu.mult)
            nc.vector.tensor_scalar(ex2, statsr[:, 1:2], 1.0 / NTOT, None,
                                    Alu.mult)
            m2 = sm.tile([C, 1], f32)
            nc.vector.tensor_tensor(m2, mean, mean, Alu.mult)
            varep = sm.tile([C, 1], f32)
            nc.vector.tensor_tensor(varep, ex2, m2, Alu.subtract)
            nc.vector.tensor_scalar(varep, varep, EPS_BN, None, Alu.add)
            inv = sm.tile([C, 1], f32)
            nc.vector.reciprocal(inv, varep)
            rstd = sm.tile([C, 1], f32)
            nc.scalar.activation(rstd, inv, Act.Sqrt)
            s_sc = sm.tile([C, 1], f32)
            nc.vector.tensor_tensor(s_sc, rstd, gb[:, 0:1], Alu.mult)
            ms = sm.tile([C, 1], f32)
            nc.vector.tensor_tensor(ms, mean, s_sc, Alu.mult)
            t_sc = sm.tile([C, 1], f32)
            nc.vector.tensor_tensor(t_sc, gb[:, 1:2], ms, Alu.subtract)

            # ---- apply BN (DVE 2x, bf16 out), write out on two queues ----
            for g in range(NGRP):
                lo, hi = g * NW, (g + 1) * NW
                ow = owp.tile([C, NW], bf16, tag="ow")
                nc.vector.tensor_scalar(ow, ypre[:, lo:hi],
                                        s_sc[:, 0:1], t_sc[:, 0:1],
                                        Alu.mult, Alu.add)
                eng = nc.sync if g % 2 == 0 else nc.scalar
                eng.dma_start(out=y_ext[:, lo:hi], in_=ow)

    _split_excess_waits(nc)
    return nc


def _fold_weights(inputs):
    f = np.float32
    W_in = inputs["w_spatial_in"].astype(np.float64)
    W_out = inputs["w_spatial_out"].astype(np.float64)
    dw_sp = inputs["w_dw_spatial"][:, 0, :].astype(np.float64)
    W_proj = inputs["w_out_proj"].astype(np.float64)
    W_mlp2 = inputs["w_mlp2"].astype(np.float64)
    dwt = float(inputs["diff_weight"])

    a_sym = dw_sp[:, 0] + dw_sp[:, 2]
    w1 = dw_sp[:, 1]
    A2 = 0.25 * W_proj @ (W_out * a_sym[None, :]) @ W_in
    B3 = W_proj @ (W_out * w1[None, :]) @ W_in + W_proj
    W_d = 0.25 * dwt * W_proj
    C2 = W_proj @ W_mlp2                     # [c, 32]
    bias_out = W_proj @ inputs["b_mlp2"].astype(np.float64)

    bf = ml_dtypes.bfloat16
    f8 = ml_dtypes.float8_e4m3
    a2t8 = (A2.T * ASC).astype(f8)
    wbf = np.concatenate(
        [B3.T.astype(bf), A2.T.astype(bf), W_d.T.astype(bf),
         np.tile(C2.T.astype(bf), (4, 1))], axis=1)
    wf32 = np.concatenate(
        [inputs["w_ch_out"].astype(f), inputs["w_ch_in"].astype(f),
         inputs["w_mlp1"].T.astype(f),
         inputs["w_ch_dw"][:, 0, :].astype(f),
         np.tile(inputs["b_mlp1"].astype(f), 4)[:, None],
         bias_out.astype(f)[:, None],
         np.stack([inputs["bn_gamma"], inputs["bn_beta"]], 1).astype(f)],
        axis=1)
    return {
        "wbf": np.ascontiguousarray(wbf),
        "a2d": np.ascontiguousarray(np.concatenate([a2t8, a2t8], axis=1)),
        "wf32": np.ascontiguousarray(wf32),
    }


def prepare_in_maps(inputs):
    wmap = _fold_weights(inputs)
    x = np.asarray(inputs["x"]).astype(np.float32)  # [B, C, H, W]
    in_maps = []
    for b in range(NCORES):
        m = dict(wmap)
        xb = x[b].reshape(C, L)
        m["x"] = np.ascontiguousarray(xb.astype(ml_dtypes.bfloat16))
        in_maps.append(m)
    return in_maps


def kernel(**inputs):
    from concourse.bass_utils import run_bass_kernel_spmd

    inputs = {k: np.asarray(v) for k, v in inputs.items()}
    if "nc" not in _CACHE:
        _CACHE["nc"] = _build_program()
    nc = _CACHE["nc"]

    in_maps = prepare_in_maps(inputs)
    res = run_bass_kernel_spmd(nc, in_maps, list(range(NCORES)))
    out = np.stack([np.asarray(res.results[b]["y"]).astype(np.float32)
                    .reshape(C, Himg, Wimg) for b in range(NCORES)])
    return out
